# revision 1
# baseline (speedup 1.0000x reference)
"""GAT layer (PyG GATConv-style) on 8 Trainium2 NeuronCores.

Strategy:
- Nodes sharded across 8 cores by destination; edges partitioned by destination
  node (per the sharding hint) in a partition-aligned layout: each destination
  node owns one SBUF partition of its block; its incoming edges sit along the
  free dim, padded to a per-block uniform length L (degree-sorted bin packing
  keeps padding ~1-2%).
- Since h[src] = x[src] @ W.T is linear, the source-feature exchange is done by
  expanding x[src] per edge slot on the host (sharding-time data movement);
  the device computes per-edge [h | a_src] with one matmul against the folded
  weight matrix Wt_ext = [W.T | W.T @ att_src-blockdiag]. No device-side
  gather or collective is needed.
- Segment softmax over incoming edges is then core-local: a_dst is a
  per-partition broadcast, exp(leaky(z)) = max(exp(z), exp(0.2 z)) on ACT,
  and the weighted aggregation is a PSUM-accumulated identity matmul.

kernel(**inputs) takes FULL inputs and returns the FULL [N, 64] output.
"""

import numpy as np
import ml_dtypes

import concourse.bass as bass
import concourse.bacc as bacc
import concourse.tile as tile
from concourse import mybir
from concourse.bass_utils import run_bass_kernel_spmd
from concourse.masks import make_identity

# Problem shape (hardcoded per contract)
N, F, E = 100000, 256, 1600000
H, C = 8, 8
HC = H * C  # 64
NEG_SLOPE = 0.2

P = 128
NCORES = 8
NB = 98                      # blocks per core
NPC = NB * P                 # 12544 node slots per core
NSLOT = NCORES * NPC         # 100352 >= N
TD = HC + H                  # 72: [h(64) | a_src(8)]
PAD_ASRC = -200.0

bf16 = ml_dtypes.bfloat16


def _host_prep(x, edge_index, W, att_src, att_dst, bias):
    src = np.asarray(edge_index[0], dtype=np.int64)
    dst = np.asarray(edge_index[1], dtype=np.int64)
    loop = np.arange(N, dtype=np.int64)
    src = np.concatenate([src, loop])
    dst = np.concatenate([dst, loop])

    deg = np.bincount(dst, minlength=N).astype(np.int64)

    # nodes sorted by degree desc -> global 128-slot blocks dealt round-robin
    # to cores so every core's j-th block has (nearly) equal max degree.
    order = np.argsort(-deg, kind="stable")
    ks = np.arange(NSLOT)
    g = ks // P
    p = ks % P
    c = g % NCORES
    j = g // NCORES
    rows = c * NPC + j * P + p          # device row of global sorted slot k
    row2node = np.full(NSLOT, -1, dtype=np.int64)
    row2node[rows[:N]] = order
    node2row = np.empty(N, dtype=np.int64)
    node2row[order] = rows[:N]

    # per-core-block uniform L schedule (exact max over the 8-block group)
    deg_slot = np.zeros(NSLOT, dtype=np.int64)
    deg_slot[:N] = deg[order]           # degree of global sorted slot k
    degb = deg_slot.reshape(NSLOT // P, P).max(axis=1)   # per global block g
    L_sched = degb.reshape(NB, NCORES).max(axis=1)
    L_sched = np.maximum(L_sched, 1)
    off = np.zeros(NB + 1, dtype=np.int64)
    off[1:] = np.cumsum(P * L_sched)
    S = int(off[-1])                    # edge slots per core

    # folded weights
    Wt = np.asarray(W, dtype=np.float64).T            # [256, 64]
    att_s = np.asarray(att_src, np.float64)           # [8, 8]
    att_d = np.asarray(att_dst, np.float64)
    Wts = np.stack([Wt[:, h * C:(h + 1) * C] @ att_s[h] for h in range(H)], axis=1)  # [256, 8]
    Wtd = np.stack([Wt[:, h * C:(h + 1) * C] @ att_d[h] for h in range(H)], axis=1)  # [256, 8]
    Wt_ext = np.concatenate([Wt, Wts], axis=1)        # [256, 72]

    # padding-slot input vector: v @ Wts = PAD_ASRC for every head. (h(v) is
    # then nonzero but bounded; exp(PAD_ASRC * slope) kills its contribution.)
    tgt = np.full(H, PAD_ASRC)
    v_pad, *_ = np.linalg.lstsq(Wts.T, tgt, rcond=None)      # [256]
    assert np.abs(Wts.T @ v_pad - tgt).max() < 1e-6

    # edge -> slot (vectorized); slot storage order (j, l, p)
    eorder = np.argsort(dst, kind="stable")
    dst_s = dst[eorder]
    src_s = src[eorder]
    starts = np.zeros(N + 1, dtype=np.int64)
    starts[1:] = np.cumsum(deg)
    l_rank = np.arange(len(dst_s), dtype=np.int64) - starts[dst_s]
    r = node2row[dst_s]
    ec = r // NPC
    within = r % NPC
    ej = within // P
    ep = within % P
    pos = off[ej] + l_rank * P + ep

    x_bf = np.asarray(x, np.float32).astype(bf16)
    v_bf = v_pad.astype(np.float32).astype(bf16)

    # per-core expanded inputs
    in_maps = []
    Wt_ext_bf = np.ascontiguousarray(Wt_ext.astype(np.float32).astype(bf16))
    Wtd_bf = np.ascontiguousarray(Wtd.astype(np.float32).astype(bf16))
    bias_rep = np.tile(np.asarray(bias, np.float32).reshape(1, HC), (P, 1))
    for cc in range(NCORES):
        m = ec == cc
        xe = np.broadcast_to(v_bf, (S, F)).copy()     # pad slots -> v_pad
        xe[pos[m]] = x_bf[src_s[m]]
        # lhsT layout per block: [L][k][128f][128slots]
        parts = []
        for jj in range(NB):
            Lj = int(L_sched[jj])
            a = xe[off[jj]:off[jj + 1]].reshape(Lj, P, F)      # [l, p, f]
            a = a.transpose(2, 0, 1)                           # [f, l, p] = [k,f128,l,p]
            parts.append(np.ascontiguousarray(a).reshape(-1))
        xeT = np.concatenate(parts)
        del xe

        # own-node x, transposed, for a_dst (bf16)
        rr = row2node[cc * NPC:(cc + 1) * NPC]
        mm = rr >= 0
        xo = np.zeros((NPC, F), dtype=bf16)
        xo[mm] = x_bf[rr[mm]]
        xoT = np.ascontiguousarray(xo.T)              # [256, NPC]

        in_maps.append({
            "xeT": xeT,
            "xoT": xoT,
            "Wt_ext": Wt_ext_bf,
            "Wtd": Wtd_bf,
            "bias_rep": bias_rep,
        })
    return in_maps, L_sched, S, row2node


def _build_program(L_sched, S, reps=1, BX=2, BH=4, BA=2, B2=3, NG=3):
    nc = bacc.Bacc("TRN2", target_bir_lowering=False, debug=False,
                   enable_asserts=False, num_devices=NCORES)
    dt = mybir.dt

    xeT = nc.dram_tensor("xeT", [S * 2 * P], dt.bfloat16, kind="ExternalInput").ap()
    xoT = nc.dram_tensor("xoT", [F, NPC], dt.bfloat16, kind="ExternalInput").ap()
    Wt_ext = nc.dram_tensor("Wt_ext", [F, TD], dt.bfloat16, kind="ExternalInput").ap()
    Wtd = nc.dram_tensor("Wtd", [F, H], dt.bfloat16, kind="ExternalInput").ap()
    bias_rep = nc.dram_tensor("bias_rep", [P, HC], dt.float32, kind="ExternalInput").ap()
    out = nc.dram_tensor("out", [NPC, HC], dt.float32, kind="ExternalOutput").ap()

    AF = mybir.ActivationFunctionType
    OP = mybir.AluOpType
    GP7 = 7  # l-groups of 7 share one 504-col psum bank

    with tile.TileContext(nc) as tc:
        with (
            tc.tile_pool(name="const", bufs=1) as constp,
            tc.tile_pool(name="resid", bufs=1) as residp,
        ):
            wt0 = constp.tile([P, TD], dt.bfloat16)
            nc.sync.dma_start(wt0[:], Wt_ext[0:P, :])
            wt1 = constp.tile([P, TD], dt.bfloat16)
            nc.sync.dma_start(wt1[:], Wt_ext[P:2 * P, :])
            wtd0 = constp.tile([P, H], dt.bfloat16)
            nc.sync.dma_start(wtd0[:], Wtd[0:P, :])
            wtd1 = constp.tile([P, H], dt.bfloat16)
            nc.sync.dma_start(wtd1[:], Wtd[P:2 * P, :])
            bias_t = constp.tile([P, HC], dt.float32)
            nc.sync.dma_start(bias_t[:], bias_rep[:])
            ident = constp.tile([P, P], dt.bfloat16)
            make_identity(nc, ident[:])

            adst_own = residp.tile([P, NB * H], dt.bfloat16)

            for _rep in range(reps):

                # ---------------- phase 1: a_dst for own nodes ----------------
                with (
                    tc.tile_pool(name="p1", bufs=3) as p1,
                    tc.tile_pool(name="p1ps", bufs=2, space="PSUM") as p1ps,
                ):
                    for t in range(NB):
                        xt0 = p1.tile([P, P], dt.bfloat16, tag="xt0")
                        nc.sync.dma_start(xt0[:], xoT[0:P, t * P:(t + 1) * P])
                        xt1 = p1.tile([P, P], dt.bfloat16, tag="xt1")
                        nc.sync.dma_start(xt1[:], xoT[P:2 * P, t * P:(t + 1) * P])
                        aps = p1ps.tile([P, H], dt.float32, space="PSUM")
                        nc.tensor.matmul(aps[:], lhsT=xt0[:], rhs=wtd0[:], start=True, stop=False)
                        nc.tensor.matmul(aps[:], lhsT=xt1[:], rhs=wtd1[:], start=False, stop=True)
                        with nc.allow_low_precision(reason="bf16 a_dst store; fp32 accum in PSUM"):
                            nc.vector.tensor_copy(out=adst_own[:, t * H:(t + 1) * H], in_=aps[:])

                # ---------------- phase 2: edge blocks ----------------
                with (
                    tc.tile_pool(name="p2x", bufs=BX) as p2x,
                    tc.tile_pool(name="p2", bufs=B2) as p2,
                    tc.tile_pool(name="p2f", bufs=2) as p2f,
                    tc.tile_pool(name="heps", bufs=BH, space="PSUM") as hepsp,
                    tc.tile_pool(name="aggps", bufs=BA, space="PSUM") as aggpsp,
                ):
                    xoff = 0
                    CH = NG * GP7  # l-chunk: at most NG heps psum banks live at a time
                    for jb in range(NB):
                        L = int(L_sched[jb])
                        xta = p2x.tile([P, L * P], dt.bfloat16, tag="xta")
                        nc.sync.dma_start(xta[:], xeT[xoff:xoff + P * L * P].rearrange("(a b) -> a b", b=L * P))
                        xoff += P * L * P
                        xtb = p2x.tile([P, L * P], dt.bfloat16, tag="xtb")
                        nc.sync.dma_start(xtb[:], xeT[xoff:xoff + P * L * P].rearrange("(a b) -> a b", b=L * P))
                        xoff += P * L * P
                        agg = aggpsp.tile([P, TD], dt.float32, space="PSUM", tag="agg")
                        for ch0 in range(0, L, CH):
                            cl = min(CH, L - ch0)
                            ngrp = (cl + GP7 - 1) // GP7
                            heps = []
                            for grp in range(ngrp):
                                gl = min(GP7, cl - grp * GP7)
                                ps = hepsp.tile([P, gl * TD], dt.float32, space="PSUM", tag="heps")
                                heps.append((ps, gl))
                            logits = p2.tile([P, cl, H], dt.float32, tag="logits")
                            for grp, (ps, gl) in enumerate(heps):
                                for li in range(gl):
                                    l = ch0 + grp * GP7 + li
                                    nc.tensor.matmul(ps[:, li * TD:(li + 1) * TD],
                                                     lhsT=xta[:, l * P:(l + 1) * P], rhs=wt0[:],
                                                     start=(li == 0), stop=False, skip_group_check=True)
                                    nc.tensor.matmul(ps[:, li * TD:(li + 1) * TD],
                                                     lhsT=xtb[:, l * P:(l + 1) * P], rhs=wt1[:],
                                                     start=False, stop=(li == gl - 1), skip_group_check=True)
                                # logits[l, h] = asrc + adst (adst bcast over l)
                                nc.vector.tensor_tensor(
                                    out=logits[:, grp * GP7:grp * GP7 + gl, :],
                                    in0=ps[:].rearrange("p (l d) -> p l d", d=TD)[:, :, HC:TD],
                                    in1=adst_own[:, jb * H:(jb + 1) * H].unsqueeze(1).to_broadcast([P, gl, H]),
                                    op=OP.add)
                            # exp(leaky(z)) = max(exp(z), exp(0.2 z))
                            ex1 = p2.tile([P, cl, H], dt.bfloat16, tag="ex1")
                            nc.scalar.activation(ex1[:], logits[:], AF.Exp)
                            ex2 = p2.tile([P, cl, H], dt.bfloat16, tag="ex2")
                            nc.scalar.activation(ex2[:], logits[:], AF.Exp, scale=NEG_SLOPE)
                            w = p2.tile([P, cl, TD], dt.bfloat16, tag="w")
                            nc.vector.tensor_tensor(out=w[:, :, HC:TD], in0=ex1[:], in1=ex2[:], op=OP.max)
                            # w[:, :, 0:64] = h * expe (per-head broadcast)
                            for grp, (ps, gl) in enumerate(heps):
                                nc.vector.tensor_tensor(
                                    out=w[:, grp * GP7:grp * GP7 + gl, 0:HC].rearrange("p l (h c) -> p l h c", c=C),
                                    in0=ps[:].rearrange("p (l d) -> p l d", d=TD)[:, :, 0:HC].rearrange("p l (h c) -> p l h c", c=C),
                                    in1=w[:, grp * GP7:grp * GP7 + gl, HC:TD].unsqueeze(3).to_broadcast([P, gl, H, C]),
                                    op=OP.mult)
                            # aggregate: psum += I.T @ w_l  -> [m | s]
                            for li in range(cl):
                                l = ch0 + li
                                nc.tensor.matmul(agg[:], lhsT=ident[:], rhs=w[:, li, :],
                                                 start=(l == 0), stop=(l == L - 1), skip_group_check=True)
                        # finalize: log_softmax(m / s + bias)
                        srecip = p2f.tile([P, H], dt.float32, tag="srecip")
                        nc.vector.reciprocal(srecip[:], agg[:, HC:TD])
                        onorm = p2f.tile([P, HC], dt.float32, tag="onorm")
                        nc.vector.tensor_tensor(
                            out=onorm[:].rearrange("p (h c) -> p h c", c=C),
                            in0=agg[:, 0:HC].rearrange("p (h c) -> p h c", c=C),
                            in1=srecip[:].unsqueeze(2).to_broadcast([P, H, C]),
                            op=OP.mult)
                        ob = p2f.tile([P, HC], dt.float32, tag="ob")
                        nc.gpsimd.tensor_tensor(out=ob[:], in0=onorm[:], in1=bias_t[:], op=OP.add)
                        mx = p2f.tile([P, 1], dt.float32, tag="mx")
                        nc.vector.tensor_reduce(mx[:], ob[:], axis=mybir.AxisListType.X, op=OP.max)
                        zs = p2f.tile([P, HC], dt.float32, tag="zs")
                        nc.vector.tensor_scalar(out=zs[:], in0=ob[:], scalar1=mx[:, 0:1],
                                                scalar2=None, op0=OP.subtract)
                        exf = p2f.tile([P, HC], dt.float32, tag="exf")
                        nc.scalar.activation(exf[:], zs[:], AF.Exp)
                        sm = p2f.tile([P, 1], dt.float32, tag="sm")
                        nc.vector.tensor_reduce(sm[:], exf[:], axis=mybir.AxisListType.X, op=OP.add)
                        ln = p2f.tile([P, 1], dt.float32, tag="ln")
                        nc.scalar.activation(ln[:], sm[:], AF.Ln)
                        fin = p2f.tile([P, HC], dt.float32, tag="fin")
                        nc.vector.tensor_scalar(out=fin[:], in0=zs[:], scalar1=ln[:, 0:1],
                                                scalar2=None, op0=OP.subtract)
                        nc.sync.dma_start(out[jb * P:(jb + 1) * P, :], fin[:])

    nc.compile()
    return nc


def kernel(x, edge_index, W, att_src, att_dst, bias):
    in_maps, L_sched, S, row2node = _host_prep(x, edge_index, W, att_src, att_dst, bias)
    nc = _build_program(L_sched, S)
    res = run_bass_kernel_spmd(nc, in_maps, core_ids=list(range(NCORES)))
    out_full = np.empty((N, HC), dtype=np.float32)
    for cc in range(NCORES):
        o = res.results[cc]["out"]
        rr = row2node[cc * NPC:(cc + 1) * NPC]
        m = rr >= 0
        out_full[rr[m]] = o[m]
    return out_full



# revision 2
# speedup vs baseline: 1.7632x; 1.7632x over previous
"""GAT layer (PyG GATConv-style) on 8 Trainium2 NeuronCores.

Strategy (v2):
- Nodes sharded across 8 cores by destination; edges partitioned by destination
  node in a partition-aligned layout: each destination node owns one SBUF
  partition of its block; its incoming edges sit along the free dim, padded to
  a per-block uniform length L (degree-sorted bin packing keeps padding ~1-2%).
- Source features are expanded per edge slot on the host (sharding-time data
  movement) in fp8-e3m4; the device computes per-edge [h | a_src] with matmuls
  against bf16 folded weights (mixed-dtype PE). Padding slots are exact zeros;
  their softmax contribution (exp(leaky(a_dst)) each) is subtracted analytically
  via a host-provided per-node pad count times the device-computed
  exp(leaky(a_dst)).
- Per-block k-halves are fused into one DMA via a feature-interleaved layout
  (partition p holds features p and p+128).
- Segment softmax is core-local: exp(leaky(z)) = max(exp(z), exp(0.2 z)) keeps
  the ACT engine on a single Exp table; the log-softmax Ln is deferred to one
  batched tail pass (avoids per-block activation-table reloads).
- Weighted aggregation is a PSUM-accumulated identity matmul; the final
  [nodes, 64] result is written with a single DMA in [partition, block, col]
  layout and re-laid-out on the host.

kernel(**inputs) takes FULL inputs and returns the FULL [N, 64] output.
"""

import numpy as np
import ml_dtypes

import concourse.bass as bass
import concourse.bacc as bacc
import concourse.tile as tile
from concourse import mybir
from concourse.bass_utils import run_bass_kernel_spmd
from concourse.masks import make_identity

# Problem shape (hardcoded per contract)
N, F, E = 100000, 256, 1600000
H, C = 8, 8
HC = H * C  # 64
NEG_SLOPE = 0.2

P = 128
NCORES = 8
NB = 98                      # blocks per core
NPC = NB * P                 # 12544 node slots per core
NSLOT = NCORES * NPC         # 100352 >= N
TD = HC + H                  # 72: [h(64) | a_src(8)]
WD = TD + H                  # 80: [h | a_src | a_dst-weights col block]

bf16 = ml_dtypes.bfloat16
f8 = ml_dtypes.float8_e3m4


def _host_prep(x, edge_index, W, att_src, att_dst, bias):
    src = np.asarray(edge_index[0], dtype=np.int64)
    dst = np.asarray(edge_index[1], dtype=np.int64)
    loop = np.arange(N, dtype=np.int64)
    src = np.concatenate([src, loop])
    dst = np.concatenate([dst, loop])

    deg = np.bincount(dst, minlength=N).astype(np.int64)

    # nodes sorted by degree desc -> global 128-slot blocks dealt round-robin
    # to cores so every core's j-th block has (nearly) equal max degree.
    order = np.argsort(-deg, kind="stable")
    ks = np.arange(NSLOT)
    g = ks // P
    p = ks % P
    c = g % NCORES
    j = g // NCORES
    rows = c * NPC + j * P + p          # device row of global sorted slot k
    row2node = np.full(NSLOT, -1, dtype=np.int64)
    row2node[rows[:N]] = order
    node2row = np.empty(N, dtype=np.int64)
    node2row[order] = rows[:N]

    # per-core-block uniform L schedule (exact max over the 8-block group)
    deg_slot = np.zeros(NSLOT, dtype=np.int64)
    deg_slot[:N] = deg[order]           # degree of global sorted slot k
    degb = deg_slot.reshape(NSLOT // P, P).max(axis=1)   # per global block g
    L_sched = degb.reshape(NB, NCORES).max(axis=1)
    L_sched = np.maximum(L_sched, 1)
    off = np.zeros(NB + 1, dtype=np.int64)
    off[1:] = np.cumsum(P * L_sched)
    S = int(off[-1])                    # edge slots per core

    # pad count per device row (for the analytic softmax-denominator fix)
    deg_row = np.zeros(NSLOT, dtype=np.int64)
    deg_row[rows] = deg_slot

    # folded weights: wte[f, :] = [Wt | Wt@att_src-blockdiag | Wt@att_dst-blockdiag]
    Wt = np.asarray(W, dtype=np.float64).T            # [256, 64]
    att_s = np.asarray(att_src, np.float64)           # [8, 8]
    att_d = np.asarray(att_dst, np.float64)
    Wts = np.stack([Wt[:, h * C:(h + 1) * C] @ att_s[h] for h in range(H)], axis=1)  # [256, 8]
    Wtd = np.stack([Wt[:, h * C:(h + 1) * C] @ att_d[h] for h in range(H)], axis=1)  # [256, 8]
    Wte = np.concatenate([Wt, Wts, Wtd], axis=1)      # [256, 80]
    # feature-interleaved: partition p holds features p and p+128
    Wte_il = np.ascontiguousarray(
        Wte.reshape(2, P, WD).transpose(1, 0, 2).astype(np.float32).astype(bf16))  # [128, 2, 80]

    # edge -> slot (vectorized); slot storage order (j, l, p)
    eorder = np.argsort(dst, kind="stable")
    dst_s = dst[eorder]
    src_s = src[eorder]
    starts = np.zeros(N + 1, dtype=np.int64)
    starts[1:] = np.cumsum(deg)
    l_rank = np.arange(len(dst_s), dtype=np.int64) - starts[dst_s]
    r = node2row[dst_s]
    ec = r // NPC
    within = r % NPC
    ej = within // P
    ep = within % P
    pos = off[ej] + l_rank * P + ep

    x_f8 = np.asarray(x, np.float32).astype(f8)
    assert np.abs(np.asarray(x, np.float32)).max() < 15.0  # e3m4 range

    bias_rep = np.tile(np.asarray(bias, np.float32).reshape(1, HC), (P, 1))

    in_maps = []
    for cc in range(NCORES):
        m = ec == cc
        xe = np.zeros((S, F), dtype=f8)               # pad slots stay zero
        xe[pos[m]] = x_f8[src_s[m]]
        # per block: [L, P, F] -> [F, L*P] -> k-half interleave [128, 2, L*P]
        parts = []
        for jj in range(NB):
            Lj = int(L_sched[jj])
            a = xe[off[jj]:off[jj + 1]].reshape(Lj, P, F)      # [l, p, f]
            a = a.transpose(2, 0, 1).reshape(2, P, Lj * P)     # [kh*128f, l*p]
            a = a.transpose(1, 0, 2)                           # [128f, kh, l*p]
            parts.append(np.ascontiguousarray(a).reshape(-1))
        xeT = np.concatenate(parts)
        del xe

        # own-node x, transposed + k-half interleaved, for a_dst
        rr = row2node[cc * NPC:(cc + 1) * NPC]
        mm = rr >= 0
        xo = np.zeros((NPC, F), dtype=f8)
        xo[mm] = x_f8[rr[mm]]
        xoT = np.ascontiguousarray(
            xo.T.reshape(2, P, NPC).transpose(1, 0, 2))        # [128, 2, NPC]

        # pad slots per row: [P, NB]
        d = deg_row[cc * NPC:(cc + 1) * NPC].reshape(NB, P)
        npad = (L_sched[:, None] - d).T.astype(np.float32).astype(bf16)

        in_maps.append({
            "xeT": xeT,
            "xoT": np.ascontiguousarray(xoT.reshape(P, 2 * NPC)),
            "wte": Wte_il,
            "bias_rep": bias_rep,
            "npad": np.ascontiguousarray(npad),
        })
    return in_maps, L_sched, S, row2node


def _build_program(L_sched, S, BX=2, BH=4, BA=2, B2=3, NG=3):
    nc = bacc.Bacc("TRN2", target_bir_lowering=False, debug=False,
                   enable_asserts=False, num_devices=NCORES)
    dt = mybir.dt

    xeT = nc.dram_tensor("xeT", [S * 2 * P], dt.float8e3, kind="ExternalInput").ap()
    xoT = nc.dram_tensor("xoT", [P, 2 * NPC], dt.float8e3, kind="ExternalInput").ap()
    wte = nc.dram_tensor("wte", [P, 2, WD], dt.bfloat16, kind="ExternalInput").ap()
    bias_rep = nc.dram_tensor("bias_rep", [P, HC], dt.float32, kind="ExternalInput").ap()
    npad = nc.dram_tensor("npad", [P, NB], dt.bfloat16, kind="ExternalInput").ap()
    out = nc.dram_tensor("out", [P, NB * HC], dt.float32, kind="ExternalOutput").ap()

    AF = mybir.ActivationFunctionType
    OP = mybir.AluOpType
    GP7 = 7  # l-groups of 7 share one 504-col psum bank

    with tile.TileContext(nc) as tc:
        with (
            tc.tile_pool(name="const", bufs=1) as constp,
            tc.tile_pool(name="resid", bufs=1) as residp,
        ):
            wte_t = constp.tile([P, 2, WD], dt.bfloat16)
            nc.sync.dma_start(wte_t[:], wte[:])
            bias_t = constp.tile([P, HC], dt.float32)
            nc.sync.dma_start(bias_t[:], bias_rep[:])
            npad_t = constp.tile([P, NB], dt.bfloat16)
            nc.sync.dma_start(npad_t[:], npad[:])
            ident = constp.tile([P, P], dt.bfloat16)
            make_identity(nc, ident[:])
            xo_t = constp.tile([P, 2, NPC], dt.float8e3)
            nc.sync.dma_start(xo_t[:], xoT[:].rearrange("p (k q) -> p k q", k=2))

            adst_own = residp.tile([P, NB * H], dt.bfloat16)
            padcorr = residp.tile([P, NB * H], dt.float32)
            obuf = residp.tile([P, NB * HC], dt.float32)
            smbuf = residp.tile([P, NB], dt.float32)

            # ---------------- phase 1: a_dst + pad correction ----------------
            with (
                tc.tile_pool(name="p1", bufs=2) as p1,
                tc.tile_pool(name="p1ps", bufs=2, space="PSUM") as p1ps,
            ):
                for t in range(NB):
                    aps = p1ps.tile([P, H], dt.float32, space="PSUM")
                    nc.tensor.matmul(aps[:], lhsT=xo_t[:, 0, t * P:(t + 1) * P],
                                     rhs=wte_t[:, 0, TD:WD], start=True, stop=False)
                    nc.tensor.matmul(aps[:], lhsT=xo_t[:, 1, t * P:(t + 1) * P],
                                     rhs=wte_t[:, 1, TD:WD], start=False, stop=True)
                    with nc.allow_low_precision(reason="bf16 a_dst store"):
                        nc.vector.tensor_copy(out=adst_own[:, t * H:(t + 1) * H], in_=aps[:])
                # e_dst = exp(leaky(a_dst)); padcorr = npad * e_dst - eps
                e1 = p1.tile([P, NB * H], dt.bfloat16)
                nc.scalar.activation(e1[:], adst_own[:], AF.Exp)
                e2 = p1.tile([P, NB * H], dt.bfloat16)
                nc.scalar.activation(e2[:], adst_own[:], AF.Exp, scale=NEG_SLOPE)
                edst = p1.tile([P, NB * H], dt.bfloat16)
                nc.vector.tensor_tensor(out=edst[:], in0=e1[:], in1=e2[:], op=OP.max)
                nc.vector.tensor_tensor(
                    out=padcorr[:].rearrange("p (t h) -> p t h", h=H),
                    in0=edst[:].rearrange("p (t h) -> p t h", h=H),
                    in1=npad_t[:].unsqueeze(2).to_broadcast([P, NB, H]),
                    op=OP.mult)
                nc.vector.tensor_scalar(out=padcorr[:], in0=padcorr[:],
                                        scalar1=1e-16, scalar2=None, op0=OP.subtract)

            # ---------------- phase 2: edge blocks ----------------
            with (
                tc.tile_pool(name="p2x", bufs=BX) as p2x,
                tc.tile_pool(name="p2", bufs=B2) as p2,
                tc.tile_pool(name="p2f", bufs=2) as p2f,
                tc.tile_pool(name="heps", bufs=BH, space="PSUM") as hepsp,
                tc.tile_pool(name="aggps", bufs=BA, space="PSUM") as aggpsp,
            ):
                xoff = 0
                CH = NG * GP7  # l-chunk: at most NG heps psum banks live at a time
                for jb in range(NB):
                    L = int(L_sched[jb])
                    xta = p2x.tile([P, 2 * L * P], dt.float8e3, tag="xta")
                    nc.sync.dma_start(
                        xta[:], xeT[xoff:xoff + P * 2 * L * P].rearrange("(a b) -> a b", b=2 * L * P))
                    xoff += P * 2 * L * P
                    xv = xta[:].rearrange("p (k l q) -> p k l q", k=2, q=P)
                    agg = aggpsp.tile([P, TD], dt.float32, space="PSUM", tag="agg")
                    for ch0 in range(0, L, CH):
                        cl = min(CH, L - ch0)
                        ngrp = (cl + GP7 - 1) // GP7
                        heps = []
                        for grp in range(ngrp):
                            gl = min(GP7, cl - grp * GP7)
                            ps = hepsp.tile([P, gl * TD], dt.float32, space="PSUM", tag="heps")
                            heps.append((ps, gl))
                        logits = p2.tile([P, cl, H], dt.float32, tag="logits")
                        for grp, (ps, gl) in enumerate(heps):
                            for li in range(gl):
                                l = ch0 + grp * GP7 + li
                                nc.tensor.matmul(ps[:, li * TD:(li + 1) * TD],
                                                 lhsT=xv[:, 0, l, :], rhs=wte_t[:, 0, 0:TD],
                                                 start=(li == 0), stop=False, skip_group_check=True)
                                nc.tensor.matmul(ps[:, li * TD:(li + 1) * TD],
                                                 lhsT=xv[:, 1, l, :], rhs=wte_t[:, 1, 0:TD],
                                                 start=False, stop=(li == gl - 1), skip_group_check=True)
                            # logits[l, h] = asrc + adst (adst bcast over l)
                            nc.vector.tensor_tensor(
                                out=logits[:, grp * GP7:grp * GP7 + gl, :],
                                in0=ps[:].rearrange("p (l d) -> p l d", d=TD)[:, :, HC:TD],
                                in1=adst_own[:, jb * H:(jb + 1) * H].unsqueeze(1).to_broadcast([P, gl, H]),
                                op=OP.add)
                        # exp(leaky(z)) = max(exp(z), exp(0.2 z))
                        ex1 = p2.tile([P, cl, H], dt.bfloat16, tag="ex1")
                        nc.scalar.activation(ex1[:], logits[:], AF.Exp)
                        ex2 = p2.tile([P, cl, H], dt.bfloat16, tag="ex2")
                        nc.scalar.activation(ex2[:], logits[:], AF.Exp, scale=NEG_SLOPE)
                        w = p2.tile([P, cl, TD], dt.bfloat16, tag="w")
                        nc.vector.tensor_tensor(out=w[:, :, HC:TD], in0=ex1[:], in1=ex2[:], op=OP.max)
                        # w[:, :, 0:64] = h * expe (per-head broadcast)
                        for grp, (ps, gl) in enumerate(heps):
                            nc.vector.tensor_tensor(
                                out=w[:, grp * GP7:grp * GP7 + gl, 0:HC].rearrange("p l (h c) -> p l h c", c=C),
                                in0=ps[:].rearrange("p (l d) -> p l d", d=TD)[:, :, 0:HC].rearrange("p l (h c) -> p l h c", c=C),
                                in1=w[:, grp * GP7:grp * GP7 + gl, HC:TD].unsqueeze(3).to_broadcast([P, gl, H, C]),
                                op=OP.mult)
                        # aggregate: psum += I.T @ w_l  -> [m | s]
                        for li in range(cl):
                            l = ch0 + li
                            nc.tensor.matmul(agg[:], lhsT=ident[:], rhs=w[:, li, :],
                                             start=(l == 0), stop=(l == L - 1), skip_group_check=True)
                    # finalize: ob = (m / s_corr) + bias; sm = sum(exp(ob))
                    scorr = p2f.tile([P, H], dt.float32, tag="scorr")
                    nc.vector.tensor_tensor(out=scorr[:], in0=agg[:, HC:TD],
                                            in1=padcorr[:, jb * H:(jb + 1) * H], op=OP.subtract)
                    srecip = p2f.tile([P, H], dt.float32, tag="srecip")
                    nc.vector.reciprocal(srecip[:], scorr[:])
                    ob = obuf[:, jb * HC:(jb + 1) * HC]
                    nc.vector.tensor_tensor(
                        out=ob.rearrange("p (h c) -> p h c", c=C),
                        in0=agg[:, 0:HC].rearrange("p (h c) -> p h c", c=C),
                        in1=srecip[:].unsqueeze(2).to_broadcast([P, H, C]),
                        op=OP.mult)
                    nc.gpsimd.tensor_tensor(out=ob, in0=ob, in1=bias_t[:], op=OP.add)
                    exf = p2f.tile([P, HC], dt.float32, tag="exf")
                    nc.scalar.activation(exf[:], ob, AF.Exp)
                    nc.vector.tensor_reduce(smbuf[:, jb:jb + 1], exf[:],
                                            axis=mybir.AxisListType.X, op=OP.add)

                # tail: fin = ob - ln(sm) (single Ln table load)
                lnb = p2f.tile([P, NB], dt.float32, tag="lnb")
                nc.scalar.activation(lnb[:], smbuf[:], AF.Ln)
                nc.vector.tensor_tensor(
                    out=obuf[:].rearrange("p (t c) -> p t c", c=HC),
                    in0=obuf[:].rearrange("p (t c) -> p t c", c=HC),
                    in1=lnb[:].unsqueeze(2).to_broadcast([P, NB, HC]),
                    op=OP.subtract)
                nc.sync.dma_start(out[:], obuf[:])

    nc.compile()
    return nc


def kernel(x, edge_index, W, att_src, att_dst, bias):
    in_maps, L_sched, S, row2node = _host_prep(x, edge_index, W, att_src, att_dst, bias)
    nc = _build_program(L_sched, S)
    res = run_bass_kernel_spmd(nc, in_maps, core_ids=list(range(NCORES)))
    out_full = np.empty((N, HC), dtype=np.float32)
    for cc in range(NCORES):
        o = np.asarray(res.results[cc]["out"])          # [128, NB*HC]
        o = o.reshape(P, NB, HC).transpose(1, 0, 2).reshape(NPC, HC)
        rr = row2node[cc * NPC:(cc + 1) * NPC]
        m = rr >= 0
        out_full[rr[m]] = o[m]
    return out_full


# revision 3
# speedup vs baseline: 2.7081x; 1.5359x over previous
"""GAT layer (PyG GATConv-style) on 8 Trainium2 NeuronCores.

Strategy (v3):
- Nodes sharded across 8 cores by destination; edges partitioned by destination
  node in a partition-aligned layout: each destination node owns one SBUF
  partition of its block; its incoming edges sit along the free dim, padded to
  a per-block uniform length L (degree-sorted bin packing keeps padding ~1-2%).
- Source features are expanded per edge slot on the host (sharding-time data
  movement) in fp8-e3m4; the device computes per-edge h and a_src with matmuls
  against bf16 folded weights (mixed-dtype PE). Padding slots are exact zeros;
  their softmax contribution (exp(leaky(a_dst)) each) is subtracted analytically
  via a host-provided per-node pad count times the device-computed
  exp(leaky(a_dst)).
- Per-edge a_src accumulates in a dedicated per-block PSUM bank (up to 63 edge
  columns) so the logits add / exp / max run once per block instead of once per
  PSUM group; per-edge h fills full 2KB PSUM banks (8 edges x 64).
- exp(leaky(z)) = max(exp(z), exp(0.2 z)) keeps ACT on one Exp table; the
  whole log-softmax finalize (1/s, bias, exp, sum, ln, subtract) runs as a
  handful of batched single ops over all 98 blocks at the end; ACT does the
  per-block PSUM->SBUF moves.
- Weighted aggregation is a PSUM-accumulated identity matmul (h-part per edge,
  e-part per edge); the final result is written with one DMA in
  [partition, block, col] layout and re-laid-out on the host.

kernel(**inputs) takes FULL inputs and returns the FULL [N, 64] output.
"""

import numpy as np
import ml_dtypes

import concourse.bass as bass
import concourse.bacc as bacc
import concourse.tile as tile
from concourse import mybir
from concourse.bass_utils import run_bass_kernel_spmd
from concourse.masks import make_identity

# Problem shape (hardcoded per contract)
N, F, E = 100000, 256, 1600000
H, C = 8, 8
HC = H * C  # 64
NEG_SLOPE = 0.2

P = 128
NCORES = 8
NB = 98                      # blocks per core
NPC = NB * P                 # 12544 node slots per core
NSLOT = NCORES * NPC         # 100352 >= N
TD = HC + H                  # 72: [h(64) | e(8)] agg layout
WD = TD + H                  # 80: wte cols [h(64) | a_src(8) | a_dst(8)]

bf16 = ml_dtypes.bfloat16
f8 = ml_dtypes.float8_e3m4


def _host_prep(x, edge_index, W, att_src, att_dst, bias):
    src = np.asarray(edge_index[0], dtype=np.int64)
    dst = np.asarray(edge_index[1], dtype=np.int64)
    loop = np.arange(N, dtype=np.int64)
    src = np.concatenate([src, loop])
    dst = np.concatenate([dst, loop])

    deg = np.bincount(dst, minlength=N).astype(np.int64)

    # nodes sorted by degree desc -> global 128-slot blocks dealt round-robin
    # to cores so every core's j-th block has (nearly) equal max degree.
    order = np.argsort(-deg, kind="stable")
    ks = np.arange(NSLOT)
    g = ks // P
    p = ks % P
    c = g % NCORES
    j = g // NCORES
    rows = c * NPC + j * P + p          # device row of global sorted slot k
    row2node = np.full(NSLOT, -1, dtype=np.int64)
    row2node[rows[:N]] = order
    node2row = np.empty(N, dtype=np.int64)
    node2row[order] = rows[:N]

    # per-core-block uniform L schedule (exact max over the 8-block group)
    deg_slot = np.zeros(NSLOT, dtype=np.int64)
    deg_slot[:N] = deg[order]           # degree of global sorted slot k
    degb = deg_slot.reshape(NSLOT // P, P).max(axis=1)   # per global block g
    L_sched = degb.reshape(NB, NCORES).max(axis=1)
    L_sched = np.maximum(L_sched, 1)
    assert L_sched.max() <= 63, "a_src PSUM bank holds at most 63 edge columns"
    off = np.zeros(NB + 1, dtype=np.int64)
    off[1:] = np.cumsum(P * L_sched)
    S = int(off[-1])                    # edge slots per core

    # pad count per device row (for the analytic softmax-denominator fix)
    deg_row = np.zeros(NSLOT, dtype=np.int64)
    deg_row[rows] = deg_slot

    # folded weights: wte[f, :] = [Wt | Wt@att_src-blockdiag | Wt@att_dst-blockdiag]
    Wt = np.asarray(W, dtype=np.float64).T            # [256, 64]
    att_s = np.asarray(att_src, np.float64)           # [8, 8]
    att_d = np.asarray(att_dst, np.float64)
    Wts = np.stack([Wt[:, h * C:(h + 1) * C] @ att_s[h] for h in range(H)], axis=1)  # [256, 8]
    Wtd = np.stack([Wt[:, h * C:(h + 1) * C] @ att_d[h] for h in range(H)], axis=1)  # [256, 8]
    Wte = np.concatenate([Wt, Wts, Wtd], axis=1)      # [256, 80]
    # feature-interleaved: partition p holds features p and p+128
    Wte_il = np.ascontiguousarray(
        Wte.reshape(2, P, WD).transpose(1, 0, 2).astype(np.float32).astype(bf16))  # [128, 2, 80]

    # edge -> slot (vectorized); slot storage order (j, l, p)
    eorder = np.argsort(dst, kind="stable")
    dst_s = dst[eorder]
    src_s = src[eorder]
    starts = np.zeros(N + 1, dtype=np.int64)
    starts[1:] = np.cumsum(deg)
    l_rank = np.arange(len(dst_s), dtype=np.int64) - starts[dst_s]
    r = node2row[dst_s]
    ec = r // NPC
    within = r % NPC
    ej = within // P
    ep = within % P
    pos = off[ej] + l_rank * P + ep

    x_f8 = np.asarray(x, np.float32).astype(f8)
    assert np.abs(np.asarray(x, np.float32)).max() < 15.0  # e3m4 range

    bias_rep = np.tile(np.asarray(bias, np.float32).reshape(1, HC), (P, 1))

    in_maps = []
    for cc in range(NCORES):
        m = ec == cc
        xe = np.zeros((S, F), dtype=f8)               # pad slots stay zero
        xe[pos[m]] = x_f8[src_s[m]]
        # per block: [L, P, F] -> [F, L*P] -> k-half interleave [128, 2, L*P]
        parts = []
        for jj in range(NB):
            Lj = int(L_sched[jj])
            a = xe[off[jj]:off[jj + 1]].reshape(Lj, P, F)      # [l, p, f]
            a = a.transpose(2, 0, 1).reshape(2, P, Lj * P)     # [kh*128f, l*p]
            a = a.transpose(1, 0, 2)                           # [128f, kh, l*p]
            parts.append(np.ascontiguousarray(a).reshape(-1))
        xeT = np.concatenate(parts)
        del xe

        # own-node x, transposed + k-half interleaved, for a_dst
        rr = row2node[cc * NPC:(cc + 1) * NPC]
        mm = rr >= 0
        xo = np.zeros((NPC, F), dtype=f8)
        xo[mm] = x_f8[rr[mm]]
        xoT = np.ascontiguousarray(
            xo.T.reshape(2, P, NPC).transpose(1, 0, 2))        # [128, 2, NPC]

        # pad slots per row: [P, NB]
        d = deg_row[cc * NPC:(cc + 1) * NPC].reshape(NB, P)
        npad = (L_sched[:, None] - d).T.astype(np.float32).astype(bf16)

        in_maps.append({
            "xeT": xeT,
            "xoT": np.ascontiguousarray(xoT.reshape(P, 2 * NPC)),
            "wte": Wte_il,
            "bias_rep": bias_rep,
            "npad": np.ascontiguousarray(npad),
        })
    return in_maps, L_sched, S, row2node


def _build_program(L_sched, S, BX=5, BH=4, B2=3):
    nc = bacc.Bacc("TRN2", target_bir_lowering=False, debug=False,
                   enable_asserts=False, num_devices=NCORES)
    dt = mybir.dt

    xeT = nc.dram_tensor("xeT", [S * 2 * P], dt.float8e3, kind="ExternalInput").ap()
    xoT = nc.dram_tensor("xoT", [P, 2 * NPC], dt.float8e3, kind="ExternalInput").ap()
    wte = nc.dram_tensor("wte", [P, 2, WD], dt.bfloat16, kind="ExternalInput").ap()
    bias_rep = nc.dram_tensor("bias_rep", [P, HC], dt.float32, kind="ExternalInput").ap()
    npad = nc.dram_tensor("npad", [P, NB], dt.bfloat16, kind="ExternalInput").ap()
    out = nc.dram_tensor("out", [P, NB * HC], dt.float32, kind="ExternalOutput").ap()

    AF = mybir.ActivationFunctionType
    OP = mybir.AluOpType
    GP8 = 8   # l-group: 8 x 64 fp32 fills one 2KB PSUM bank
    PB1 = 8   # phase-1: a_dst blocks per PSUM bank

    with tile.TileContext(nc) as tc:
        with (
            tc.tile_pool(name="const", bufs=1) as constp,
            tc.tile_pool(name="resid", bufs=1) as residp,
        ):
            wte_t = constp.tile([P, 2, WD], dt.bfloat16)
            nc.sync.dma_start(wte_t[:], wte[:])
            bias_t = constp.tile([P, HC], dt.float32)
            nc.sync.dma_start(bias_t[:], bias_rep[:])
            npad_t = constp.tile([P, NB], dt.bfloat16)
            nc.sync.dma_start(npad_t[:], npad[:])
            ident = constp.tile([P, P], dt.bfloat16)
            make_identity(nc, ident[:])
            xo_t = constp.tile([P, 2, NPC], dt.float8e3)
            nc.sync.dma_start(xo_t[:], xoT[:].rearrange("p (k q) -> p k q", k=2))

            adst_own = residp.tile([P, NB * H], dt.bfloat16)
            padcorr = residp.tile([P, NB * H], dt.float32)
            aggsb = residp.tile([P, NB * TD], dt.float32)
            obuf = residp.tile([P, NB * HC], dt.float32)

            # ---------------- phase 1: a_dst + pad correction ----------------
            with (
                tc.tile_pool(name="p1", bufs=2) as p1,
                tc.tile_pool(name="p1ps", bufs=3, space="PSUM") as p1ps,
            ):
                for t0 in range(0, NB, PB1):
                    tn = min(PB1, NB - t0)
                    aps = p1ps.tile([P, tn * H], dt.float32, space="PSUM")
                    for ti in range(tn):
                        t = t0 + ti
                        nc.tensor.matmul(aps[:, ti * H:(ti + 1) * H],
                                         lhsT=xo_t[:, 0, t * P:(t + 1) * P],
                                         rhs=wte_t[:, 0, TD:WD],
                                         start=(ti == 0), stop=False, skip_group_check=True)
                        nc.tensor.matmul(aps[:, ti * H:(ti + 1) * H],
                                         lhsT=xo_t[:, 1, t * P:(t + 1) * P],
                                         rhs=wte_t[:, 1, TD:WD],
                                         start=False, stop=(ti == tn - 1), skip_group_check=True)
                    with nc.allow_low_precision(reason="bf16 a_dst store"):
                        nc.scalar.copy(out=adst_own[:, t0 * H:(t0 + tn) * H], in_=aps[:])
                # e_dst = exp(leaky(a_dst)); padcorr = npad * e_dst - eps
                e1 = p1.tile([P, NB * H], dt.bfloat16)
                nc.scalar.activation(e1[:], adst_own[:], AF.Exp)
                e2 = p1.tile([P, NB * H], dt.bfloat16)
                nc.scalar.activation(e2[:], adst_own[:], AF.Exp, scale=NEG_SLOPE)
                edst = p1.tile([P, NB * H], dt.bfloat16)
                nc.vector.tensor_tensor(out=edst[:], in0=e1[:], in1=e2[:], op=OP.max)
                nc.vector.tensor_tensor(
                    out=padcorr[:].rearrange("p (t h) -> p t h", h=H),
                    in0=edst[:].rearrange("p (t h) -> p t h", h=H),
                    in1=npad_t[:].unsqueeze(2).to_broadcast([P, NB, H]),
                    op=OP.mult)
                nc.vector.tensor_scalar(out=padcorr[:], in0=padcorr[:],
                                        scalar1=1e-16, scalar2=None, op0=OP.subtract)

            # ---------------- phase 2: edge blocks ----------------
            with (
                tc.tile_pool(name="p2x", bufs=BX) as p2x,
                tc.tile_pool(name="p2", bufs=B2) as p2,
                tc.tile_pool(name="asrcps", bufs=2, space="PSUM") as asrcp,
                tc.tile_pool(name="heps", bufs=BH, space="PSUM") as hepsp,
                tc.tile_pool(name="aggps", bufs=2, space="PSUM") as aggpsp,
            ):
                xoff = 0
                for jb in range(NB):
                    L = int(L_sched[jb])
                    xta = p2x.tile([P, 2 * L * P], dt.float8e3, tag="xta")
                    nc.sync.dma_start(
                        xta[:], xeT[xoff:xoff + P * 2 * L * P].rearrange("(a b) -> a b", b=2 * L * P))
                    xoff += P * 2 * L * P
                    xv = xta[:].rearrange("p (k l q) -> p k l q", k=2, q=P)

                    # per-edge a_src for the whole block in one PSUM bank
                    asrc = asrcp.tile([P, L * H], dt.float32, space="PSUM", tag="asrc")
                    for l in range(L):
                        nc.tensor.matmul(asrc[:, l * H:(l + 1) * H],
                                         lhsT=xv[:, 0, l, :], rhs=wte_t[:, 0, HC:TD],
                                         start=(l == 0), stop=False, skip_group_check=True)
                        nc.tensor.matmul(asrc[:, l * H:(l + 1) * H],
                                         lhsT=xv[:, 1, l, :], rhs=wte_t[:, 1, HC:TD],
                                         start=False, stop=(l == L - 1), skip_group_check=True)
                    # logits z = a_src + a_dst ; e = max(exp(z), exp(0.2 z))
                    z = p2.tile([P, L, H], dt.float32, tag="z")
                    nc.vector.tensor_tensor(
                        out=z[:],
                        in0=asrc[:].rearrange("p (l h) -> p l h", h=H),
                        in1=adst_own[:, jb * H:(jb + 1) * H].unsqueeze(1).to_broadcast([P, L, H]),
                        op=OP.add)
                    ex1 = p2.tile([P, L, H], dt.bfloat16, tag="ex1")
                    nc.scalar.activation(ex1[:], z[:], AF.Exp)
                    ex2 = p2.tile([P, L, H], dt.bfloat16, tag="ex2")
                    nc.scalar.activation(ex2[:], z[:], AF.Exp, scale=NEG_SLOPE)
                    eb = p2.tile([P, L, H], dt.bfloat16, tag="eb")
                    nc.vector.tensor_tensor(out=eb[:], in0=ex1[:], in1=ex2[:], op=OP.max)

                    agg = aggpsp.tile([P, TD], dt.float32, space="PSUM", tag="agg")
                    # e-part aggregation: agg[:, 64:72] += I.T @ e_l
                    for l in range(L):
                        nc.tensor.matmul(agg[:, HC:TD], lhsT=ident[:], rhs=eb[:, l, :],
                                         start=(l == 0), stop=False, skip_group_check=True)
                    # h-part: per 8-edge group, h matmuls -> weight -> aggregate
                    for ch0 in range(0, L, GP8):
                        gl = min(GP8, L - ch0)
                        ps = hepsp.tile([P, gl * HC], dt.float32, space="PSUM", tag="heps")
                        for li in range(gl):
                            l = ch0 + li
                            nc.tensor.matmul(ps[:, li * HC:(li + 1) * HC],
                                             lhsT=xv[:, 0, l, :], rhs=wte_t[:, 0, 0:HC],
                                             start=(li == 0), stop=False, skip_group_check=True)
                            nc.tensor.matmul(ps[:, li * HC:(li + 1) * HC],
                                             lhsT=xv[:, 1, l, :], rhs=wte_t[:, 1, 0:HC],
                                             start=False, stop=(li == gl - 1), skip_group_check=True)
                        w = p2.tile([P, gl, HC], dt.bfloat16, tag="w")
                        nc.vector.tensor_tensor(
                            out=w[:].rearrange("p l (h c) -> p l h c", c=C),
                            in0=ps[:].rearrange("p (l h c) -> p l h c", h=H, c=C),
                            in1=eb[:, ch0:ch0 + gl, :].unsqueeze(3).to_broadcast([P, gl, H, C]),
                            op=OP.mult)
                        for li in range(gl):
                            l = ch0 + li
                            nc.tensor.matmul(agg[:, 0:HC], lhsT=ident[:], rhs=w[:, li, :],
                                             start=False, stop=(l == L - 1), skip_group_check=True)
                    # park [m | s] in SBUF; all normalization is batched later
                    nc.scalar.copy(out=aggsb[:, jb * TD:(jb + 1) * TD], in_=agg[:])

                # ---------------- batched finalize ----------------
                with tc.tile_pool(name="tail", bufs=1) as tailp:
                    av = aggsb[:].rearrange("p (t d) -> p t d", d=TD)
                    sden = tailp.tile([P, NB, H], dt.float32)
                    nc.vector.tensor_tensor(out=sden[:], in0=av[:, :, HC:TD],
                                            in1=padcorr[:].rearrange("p (t h) -> p t h", h=H),
                                            op=OP.subtract)
                    srec = tailp.tile([P, NB, H], dt.float32)
                    nc.vector.reciprocal(srec[:], sden[:])
                    ov = obuf[:].rearrange("p (t d) -> p t d", d=HC)
                    nc.vector.tensor_tensor(
                        out=ov.rearrange("p t (h c) -> p t h c", c=C),
                        in0=av[:, :, 0:HC].rearrange("p t (h c) -> p t h c", c=C),
                        in1=srec[:].unsqueeze(3).to_broadcast([P, NB, H, C]),
                        op=OP.mult)
                    nc.gpsimd.tensor_tensor(
                        out=ov, in0=ov,
                        in1=bias_t[:].unsqueeze(1).to_broadcast([P, NB, HC]), op=OP.add)
                    exf = tailp.tile([P, NB, HC], dt.bfloat16)
                    nc.scalar.activation(exf[:], ov, AF.Exp)
                    sm = tailp.tile([P, NB], dt.float32)
                    nc.vector.tensor_reduce(sm[:].unsqueeze(2), exf[:],
                                            axis=mybir.AxisListType.X, op=OP.add)
                    lnb = tailp.tile([P, NB], dt.float32)
                    nc.scalar.activation(lnb[:], sm[:], AF.Ln)
                    nc.vector.tensor_tensor(
                        out=ov, in0=ov,
                        in1=lnb[:].unsqueeze(2).to_broadcast([P, NB, HC]),
                        op=OP.subtract)
                    nc.sync.dma_start(out[:], obuf[:])

    nc.compile()
    return nc


def kernel(x, edge_index, W, att_src, att_dst, bias):
    in_maps, L_sched, S, row2node = _host_prep(x, edge_index, W, att_src, att_dst, bias)
    nc = _build_program(L_sched, S)
    res = run_bass_kernel_spmd(nc, in_maps, core_ids=list(range(NCORES)))
    out_full = np.empty((N, HC), dtype=np.float32)
    for cc in range(NCORES):
        o = np.asarray(res.results[cc]["out"])          # [128, NB*HC]
        o = o.reshape(P, NB, HC).transpose(1, 0, 2).reshape(NPC, HC)
        rr = row2node[cc * NPC:(cc + 1) * NPC]
        m = rr >= 0
        out_full[rr[m]] = o[m]
    return out_full


# revision 11
# speedup vs baseline: 2.7755x; 1.0249x over previous
"""GAT layer (PyG GATConv-style) on 8 Trainium2 NeuronCores.

Strategy (v3):
- Nodes sharded across 8 cores by destination; edges partitioned by destination
  node in a partition-aligned layout: each destination node owns one SBUF
  partition of its block; its incoming edges sit along the free dim, padded to
  a per-block uniform length L (degree-sorted bin packing keeps padding ~1-2%).
- Source features are expanded per edge slot on the host (sharding-time data
  movement) in fp8-e3m4; the device computes per-edge h and a_src with matmuls
  against bf16 folded weights (mixed-dtype PE). Padding slots are exact zeros;
  their softmax contribution (exp(leaky(a_dst)) each) is subtracted analytically
  via a host-provided per-node pad count times the device-computed
  exp(leaky(a_dst)).
- Per-edge a_src accumulates in a dedicated per-block PSUM bank (up to 63 edge
  columns) so the logits add / exp / max run once per block instead of once per
  PSUM group; per-edge h fills full 2KB PSUM banks (8 edges x 64).
- exp(leaky(z)) = max(exp(z), exp(0.2 z)) keeps ACT on one Exp table; the
  whole log-softmax finalize (1/s, bias, exp, sum, ln, subtract) runs as a
  handful of batched single ops over all 98 blocks at the end; ACT does the
  per-block PSUM->SBUF moves.
- Weighted aggregation is a PSUM-accumulated identity matmul (h-part per edge,
  e-part per edge); the final result is written with one DMA in
  [partition, block, col] layout and re-laid-out on the host.

kernel(**inputs) takes FULL inputs and returns the FULL [N, 64] output.
"""

import numpy as np
import ml_dtypes

import concourse.bass as bass
import concourse.bacc as bacc
import concourse.tile as tile
from concourse import mybir
from concourse.bass_utils import run_bass_kernel_spmd
from concourse.masks import make_identity

# Problem shape (hardcoded per contract)
N, F, E = 100000, 256, 1600000
H, C = 8, 8
HC = H * C  # 64
NEG_SLOPE = 0.2

P = 128
NCORES = 8
NB = 98                      # blocks per core
NPC = NB * P                 # 12544 node slots per core
NSLOT = NCORES * NPC         # 100352 >= N
TD = HC + H                  # 72: [h(64) | e(8)] agg layout
WD = TD + H                  # 80: wte cols [h(64) | a_src(8) | a_dst(8)]

bf16 = ml_dtypes.bfloat16
f8 = ml_dtypes.float8_e3m4


def _host_prep(x, edge_index, W, att_src, att_dst, bias):
    src = np.asarray(edge_index[0], dtype=np.int64)
    dst = np.asarray(edge_index[1], dtype=np.int64)
    loop = np.arange(N, dtype=np.int64)
    src = np.concatenate([src, loop])
    dst = np.concatenate([dst, loop])

    deg = np.bincount(dst, minlength=N).astype(np.int64)

    # nodes sorted by degree desc -> global 128-slot blocks dealt round-robin
    # to cores so every core's j-th block has (nearly) equal max degree.
    order = np.argsort(-deg, kind="stable")
    ks = np.arange(NSLOT)
    g = ks // P
    p = ks % P
    c = g % NCORES
    j = g // NCORES
    rows = c * NPC + j * P + p          # device row of global sorted slot k
    row2node = np.full(NSLOT, -1, dtype=np.int64)
    row2node[rows[:N]] = order
    node2row = np.empty(N, dtype=np.int64)
    node2row[order] = rows[:N]

    # per-core-block uniform L schedule (exact max over the 8-block group)
    deg_slot = np.zeros(NSLOT, dtype=np.int64)
    deg_slot[:N] = deg[order]           # degree of global sorted slot k
    degb = deg_slot.reshape(NSLOT // P, P).max(axis=1)   # per global block g
    L_sched = degb.reshape(NB, NCORES).max(axis=1)
    L_sched = np.maximum(L_sched, 1)
    assert L_sched.max() <= 63, "a_src PSUM bank holds at most 63 edge columns"
    off = np.zeros(NB + 1, dtype=np.int64)
    off[1:] = np.cumsum(P * L_sched)
    S = int(off[-1])                    # edge slots per core

    # pad count per device row (for the analytic softmax-denominator fix)
    deg_row = np.zeros(NSLOT, dtype=np.int64)
    deg_row[rows] = deg_slot

    # folded weights: wte[f, :] = [Wt | Wt@att_src-blockdiag | Wt@att_dst-blockdiag]
    Wt = np.asarray(W, dtype=np.float64).T            # [256, 64]
    att_s = np.asarray(att_src, np.float64)           # [8, 8]
    att_d = np.asarray(att_dst, np.float64)
    Wts = np.stack([Wt[:, h * C:(h + 1) * C] @ att_s[h] for h in range(H)], axis=1)  # [256, 8]
    Wtd = np.stack([Wt[:, h * C:(h + 1) * C] @ att_d[h] for h in range(H)], axis=1)  # [256, 8]
    Wte = np.concatenate([Wt, Wts, Wtd], axis=1)      # [256, 80]
    # feature-interleaved: partition p holds features p and p+128
    Wte_il = np.ascontiguousarray(
        Wte.reshape(2, P, WD).transpose(1, 0, 2).astype(np.float32).astype(bf16))  # [128, 2, 80]

    # edge -> slot (vectorized); slot storage order (j, l, p)
    eorder = np.argsort(dst, kind="stable")
    dst_s = dst[eorder]
    src_s = src[eorder]
    starts = np.zeros(N + 1, dtype=np.int64)
    starts[1:] = np.cumsum(deg)
    l_rank = np.arange(len(dst_s), dtype=np.int64) - starts[dst_s]
    r = node2row[dst_s]
    ec = r // NPC
    within = r % NPC
    ej = within // P
    ep = within % P
    pos = off[ej] + l_rank * P + ep

    x_f8 = np.asarray(x, np.float32).astype(f8)
    assert np.abs(np.asarray(x, np.float32)).max() < 15.0  # e3m4 range

    bias_rep = np.tile(np.asarray(bias, np.float32).reshape(1, HC), (P, 1))

    in_maps = []
    for cc in range(NCORES):
        m = ec == cc
        xe = np.zeros((S, F), dtype=f8)               # pad slots stay zero
        xe[pos[m]] = x_f8[src_s[m]]
        # per block: [L, P, F] -> [F, L*P] -> k-half interleave [128, 2, L*P]
        parts = []
        for jj in range(NB):
            Lj = int(L_sched[jj])
            a = xe[off[jj]:off[jj + 1]].reshape(Lj, P, F)      # [l, p, f]
            a = a.transpose(2, 0, 1).reshape(2, P, Lj * P)     # [kh*128f, l*p]
            a = a.transpose(1, 0, 2)                           # [128f, kh, l*p]
            parts.append(np.ascontiguousarray(a).reshape(-1))
        xeT = np.concatenate(parts)
        del xe

        # own-node x, transposed + k-half interleaved, for a_dst
        rr = row2node[cc * NPC:(cc + 1) * NPC]
        mm = rr >= 0
        xo = np.zeros((NPC, F), dtype=f8)
        xo[mm] = x_f8[rr[mm]]
        xoT = np.ascontiguousarray(
            xo.T.reshape(2, P, NPC).transpose(1, 0, 2))        # [128, 2, NPC]

        # pad slots per row: [P, NB]
        d = deg_row[cc * NPC:(cc + 1) * NPC].reshape(NB, P)
        npad = (L_sched[:, None] - d).T.astype(np.float32).astype(bf16)

        in_maps.append({
            "xeT": xeT,
            "xoT": np.ascontiguousarray(xoT.reshape(P, 2 * NPC)),
            "wte": Wte_il,
            "bias_rep": bias_rep,
            "npad": np.ascontiguousarray(npad),
        })
    return in_maps, L_sched, S, row2node


def _build_program(L_sched, S, BX=5, BH=4, B2=3):
    nc = bacc.Bacc("TRN2", target_bir_lowering=False, debug=False,
                   enable_asserts=False, num_devices=NCORES)
    dt = mybir.dt

    xeT = nc.dram_tensor("xeT", [S * 2 * P], dt.float8e3, kind="ExternalInput").ap()
    xoT = nc.dram_tensor("xoT", [P, 2 * NPC], dt.float8e3, kind="ExternalInput").ap()
    wte = nc.dram_tensor("wte", [P, 2, WD], dt.bfloat16, kind="ExternalInput").ap()
    bias_rep = nc.dram_tensor("bias_rep", [P, HC], dt.float32, kind="ExternalInput").ap()
    npad = nc.dram_tensor("npad", [P, NB], dt.bfloat16, kind="ExternalInput").ap()
    out = nc.dram_tensor("out", [P, NB * HC], dt.float32, kind="ExternalOutput").ap()

    AF = mybir.ActivationFunctionType
    OP = mybir.AluOpType
    GP8 = 8   # l-group: 8 x 64 fp32 fills one 2KB PSUM bank
    PB1 = 8   # phase-1: a_dst blocks per PSUM bank
    GB = 14   # finalize group: blocks normalized together, overlapping phase 2

    with tile.TileContext(nc) as tc:
        with (
            tc.tile_pool(name="const", bufs=1) as constp,
            tc.tile_pool(name="resid", bufs=1) as residp,
        ):
            wte_t = constp.tile([P, 2, WD], dt.bfloat16)
            nc.sync.dma_start(wte_t[:], wte[:])
            bias_t = constp.tile([P, HC], dt.float32)
            nc.sync.dma_start(bias_t[:], bias_rep[:])
            npad_t = constp.tile([P, NB], dt.bfloat16)
            nc.sync.dma_start(npad_t[:], npad[:])
            ident = constp.tile([P, P], dt.bfloat16)
            make_identity(nc, ident[:])
            xo_t = constp.tile([P, 2, NPC], dt.float8e3)
            nc.sync.dma_start(xo_t[:], xoT[:].rearrange("p (k q) -> p k q", k=2))

            adst_own = residp.tile([P, NB * H], dt.bfloat16)
            padcorr = residp.tile([P, NB * H], dt.float32)
            aggsb = residp.tile([P, NB * TD], dt.float32)
            obuf = residp.tile([P, NB * HC], dt.float32)
            smbuf = residp.tile([P, NB], dt.float32)

            # ---------------- phase 1: a_dst + pad correction ----------------
            with (
                tc.tile_pool(name="p1", bufs=2) as p1,
                tc.tile_pool(name="p1ps", bufs=3, space="PSUM") as p1ps,
            ):
                for t0 in range(0, NB, PB1):
                    tn = min(PB1, NB - t0)
                    aps = p1ps.tile([P, tn * H], dt.float32, space="PSUM")
                    for ti in range(tn):
                        t = t0 + ti
                        nc.tensor.matmul(aps[:, ti * H:(ti + 1) * H],
                                         lhsT=xo_t[:, 0, t * P:(t + 1) * P],
                                         rhs=wte_t[:, 0, TD:WD],
                                         start=(ti == 0), stop=False, skip_group_check=True)
                        nc.tensor.matmul(aps[:, ti * H:(ti + 1) * H],
                                         lhsT=xo_t[:, 1, t * P:(t + 1) * P],
                                         rhs=wte_t[:, 1, TD:WD],
                                         start=False, stop=(ti == tn - 1), skip_group_check=True)
                    with nc.allow_low_precision(reason="bf16 a_dst store"):
                        nc.scalar.copy(out=adst_own[:, t0 * H:(t0 + tn) * H], in_=aps[:])
                # e_dst = exp(leaky(a_dst)); padcorr = npad * e_dst - eps
                e1 = p1.tile([P, NB * H], dt.bfloat16)
                nc.scalar.activation(e1[:], adst_own[:], AF.Exp)
                e2 = p1.tile([P, NB * H], dt.bfloat16)
                nc.scalar.activation(e2[:], adst_own[:], AF.Exp, scale=NEG_SLOPE)
                edst = p1.tile([P, NB * H], dt.bfloat16)
                nc.vector.tensor_tensor(out=edst[:], in0=e1[:], in1=e2[:], op=OP.max)
                nc.vector.tensor_tensor(
                    out=padcorr[:].rearrange("p (t h) -> p t h", h=H),
                    in0=edst[:].rearrange("p (t h) -> p t h", h=H),
                    in1=npad_t[:].unsqueeze(2).to_broadcast([P, NB, H]),
                    op=OP.mult)
                nc.vector.tensor_scalar(out=padcorr[:], in0=padcorr[:],
                                        scalar1=1e-16, scalar2=None, op0=OP.subtract)

            # ---------------- phase 2: edge blocks ----------------
            with (
                tc.tile_pool(name="p2x", bufs=BX) as p2x,
                tc.tile_pool(name="p2", bufs=B2) as p2,
                tc.tile_pool(name="p2f", bufs=2) as p2f,
                tc.tile_pool(name="asrcps", bufs=2, space="PSUM") as asrcp,
                tc.tile_pool(name="heps", bufs=BH, space="PSUM") as hepsp,
                tc.tile_pool(name="aggps", bufs=2, space="PSUM") as aggpsp,
            ):
                xoff = 0
                for jb in range(NB):
                    L = int(L_sched[jb])
                    xta = p2x.tile([P, 2 * L * P], dt.float8e3, tag="xta")
                    nc.sync.dma_start(
                        xta[:], xeT[xoff:xoff + P * 2 * L * P].rearrange("(a b) -> a b", b=2 * L * P))
                    xoff += P * 2 * L * P
                    xv = xta[:].rearrange("p (k l q) -> p k l q", k=2, q=P)

                    # per-edge a_src for the whole block in one PSUM bank
                    asrc = asrcp.tile([P, L * H], dt.float32, space="PSUM", tag="asrc")
                    for l in range(L):
                        nc.tensor.matmul(asrc[:, l * H:(l + 1) * H],
                                         lhsT=xv[:, 0, l, :], rhs=wte_t[:, 0, HC:TD],
                                         start=(l == 0), stop=False, skip_group_check=True)
                        nc.tensor.matmul(asrc[:, l * H:(l + 1) * H],
                                         lhsT=xv[:, 1, l, :], rhs=wte_t[:, 1, HC:TD],
                                         start=False, stop=(l == L - 1), skip_group_check=True)
                    # logits z = a_src + a_dst ; e = max(exp(z), exp(0.2 z))
                    z = p2.tile([P, L, H], dt.float32, tag="z")
                    nc.vector.tensor_tensor(
                        out=z[:],
                        in0=asrc[:].rearrange("p (l h) -> p l h", h=H),
                        in1=adst_own[:, jb * H:(jb + 1) * H].unsqueeze(1).to_broadcast([P, L, H]),
                        op=OP.add)
                    ex1 = p2.tile([P, L, H], dt.bfloat16, tag="ex1")
                    nc.scalar.activation(ex1[:], z[:], AF.Exp)
                    ex2 = p2.tile([P, L, H], dt.bfloat16, tag="ex2")
                    nc.scalar.activation(ex2[:], z[:], AF.Exp, scale=NEG_SLOPE)
                    eb = p2.tile([P, L, H], dt.bfloat16, tag="eb")
                    nc.vector.tensor_tensor(out=eb[:], in0=ex1[:], in1=ex2[:], op=OP.max)

                    agg = aggpsp.tile([P, TD], dt.float32, space="PSUM", tag="agg")
                    # e-part aggregation: agg[:, 64:72] += I.T @ e_l
                    for l in range(L):
                        nc.tensor.matmul(agg[:, HC:TD], lhsT=ident[:], rhs=eb[:, l, :],
                                         start=(l == 0), stop=False, skip_group_check=True)
                    # h-part: per 8-edge group, h matmuls -> weight -> aggregate
                    for ch0 in range(0, L, GP8):
                        gl = min(GP8, L - ch0)
                        ps = hepsp.tile([P, gl * HC], dt.float32, space="PSUM", tag="heps")
                        for li in range(gl):
                            l = ch0 + li
                            nc.tensor.matmul(ps[:, li * HC:(li + 1) * HC],
                                             lhsT=xv[:, 0, l, :], rhs=wte_t[:, 0, 0:HC],
                                             start=(li == 0), stop=False, skip_group_check=True)
                            nc.tensor.matmul(ps[:, li * HC:(li + 1) * HC],
                                             lhsT=xv[:, 1, l, :], rhs=wte_t[:, 1, 0:HC],
                                             start=False, stop=(li == gl - 1), skip_group_check=True)
                        w = p2.tile([P, gl, HC], dt.bfloat16, tag="w")
                        nc.vector.tensor_tensor(
                            out=w[:].rearrange("p l (h c) -> p l h c", c=C),
                            in0=ps[:].rearrange("p (l h c) -> p l h c", h=H, c=C),
                            in1=eb[:, ch0:ch0 + gl, :].unsqueeze(3).to_broadcast([P, gl, H, C]),
                            op=OP.mult)
                        for li in range(gl):
                            l = ch0 + li
                            nc.tensor.matmul(agg[:, 0:HC], lhsT=ident[:], rhs=w[:, li, :],
                                             start=False, stop=(l == L - 1), skip_group_check=True)
                    # park [m | s] in SBUF; normalization runs in block groups
                    nc.scalar.copy(out=aggsb[:, jb * TD:(jb + 1) * TD], in_=agg[:])

                    # ---------------- grouped finalize (overlaps phase 2) ----------------
                    if (jb + 1) % GB == 0:
                        g0 = jb + 1 - GB
                        av = aggsb[:, g0 * TD:(jb + 1) * TD].rearrange("p (t d) -> p t d", d=TD)
                        pv = padcorr[:, g0 * H:(jb + 1) * H].rearrange("p (t h) -> p t h", h=H)
                        sden = p2f.tile([P, GB, H], dt.float32, tag="sden")
                        nc.gpsimd.tensor_tensor(out=sden[:], in0=av[:, :, HC:TD],
                                                in1=pv, op=OP.subtract)
                        srec = p2f.tile([P, GB, H], dt.float32, tag="srec")
                        nc.vector.reciprocal(srec[:], sden[:])
                        ov = obuf[:, g0 * HC:(jb + 1) * HC].rearrange("p (t d) -> p t d", d=HC)
                        nc.gpsimd.tensor_tensor(
                            out=ov.rearrange("p t (h c) -> p t h c", c=C),
                            in0=av[:, :, 0:HC].rearrange("p t (h c) -> p t h c", c=C),
                            in1=srec[:].unsqueeze(3).to_broadcast([P, GB, H, C]),
                            op=OP.mult)
                        nc.gpsimd.tensor_tensor(
                            out=ov, in0=ov,
                            in1=bias_t[:].unsqueeze(1).to_broadcast([P, GB, HC]), op=OP.add)
                        exf = p2f.tile([P, GB, HC], dt.bfloat16, tag="exf")
                        nc.scalar.activation(exf[:], ov, AF.Exp)
                        nc.vector.tensor_reduce(
                            smbuf[:, g0:jb + 1].unsqueeze(2), exf[:],
                            axis=mybir.AxisListType.X, op=OP.add)

                # ---------------- deferred log-softmax tail ----------------
                with tc.tile_pool(name="tail", bufs=1) as tailp:
                    lnb = tailp.tile([P, NB], dt.float32)
                    nc.scalar.activation(lnb[:], smbuf[:], AF.Ln)
                    ovv = obuf[:].rearrange("p (t d) -> p t d", d=HC)
                    nc.vector.tensor_tensor(
                        out=ovv, in0=ovv,
                        in1=lnb[:].unsqueeze(2).to_broadcast([P, NB, HC]),
                        op=OP.subtract)
                    nc.sync.dma_start(out[:], obuf[:])

    nc.compile()
    return nc


def kernel(x, edge_index, W, att_src, att_dst, bias):
    in_maps, L_sched, S, row2node = _host_prep(x, edge_index, W, att_src, att_dst, bias)
    nc = _build_program(L_sched, S)
    res = run_bass_kernel_spmd(nc, in_maps, core_ids=list(range(NCORES)))
    out_full = np.empty((N, HC), dtype=np.float32)
    for cc in range(NCORES):
        o = np.asarray(res.results[cc]["out"])          # [128, NB*HC]
        o = o.reshape(P, NB, HC).transpose(1, 0, 2).reshape(NPC, HC)
        rr = row2node[cc * NPC:(cc + 1) * NPC]
        m = rr >= 0
        out_full[rr[m]] = o[m]
    return out_full


# revision 12
# speedup vs baseline: 2.9878x; 1.0765x over previous
"""GAT layer (PyG GATConv-style) on 8 Trainium2 NeuronCores.

Strategy (v5):
- Nodes sharded across 8 cores by destination; edges partitioned by destination
  node in a partition-aligned layout (degree-sorted bin packing, ~1.3% pad).
- Host expands x[src] per edge slot in fp8-e4m3 with a k-half-interleaved
  layout [pf][kh][l][p] that doubles as the DoubleRow matmul operand layout;
  per-edge h uses ONE DoubleRow fp8 matmul (k=256 in one pass, 0.5 cyc/row),
  as does per-edge a_src. Folded weights are fp8-e4m3, scaled x4 to dodge
  subnormals; the 1/4 is folded into the exp scale (attention) and the
  1/s normalization (h path).
- a_dst is pre-filled into the per-block a_src PSUM bank by a matmul with a
  broadcast-tiled Wtd rhs, so logits need no separate add; exps read PSUM.
- Padding slots are exact zeros; their softmax contribution is subtracted
  analytically (host pad-count x device exp(leaky(a_dst))).
- exp(leaky(z)) = max(exp(z), exp(0.2 z)) keeps ACT on one Exp table; the
  log-softmax finalize runs in 14-block groups overlapped under phase 2
  (Pool does the fp32 elementwise, ACT the exp and PSUM->SBUF moves), with
  only Ln + subtract + one fp16 output DMA at the end.
- Weighted aggregation: PSUM-accumulated DoubleRow identity matmuls over
  l-pairs (fp8 messages), bf16 identity for odd leftovers.

kernel(**inputs) takes FULL inputs and returns the FULL [N, 64] output.
"""

import numpy as np
import ml_dtypes

import concourse.bass as bass
import concourse.bacc as bacc
import concourse.tile as tile
from concourse import mybir
from concourse.bass_utils import run_bass_kernel_spmd
from concourse.masks import make_identity

# Problem shape (hardcoded per contract)
N, F, E = 100000, 256, 1600000
H, C = 8, 8
HC = H * C  # 64
NEG_SLOPE = 0.2
SC_H = 4.0   # fp8 weight scale for the h projection
SC_A = 4.0   # fp8/bf16 weight scale for the attention projections

P = 128
NCORES = 8
NB = 98                      # blocks per core
NPC = NB * P                 # 12544 node slots per core
NSLOT = NCORES * NPC         # 100352 >= N
TD = HC + H                  # 72: agg layout [h(64) | e(8)]

bf16 = ml_dtypes.bfloat16
f8 = ml_dtypes.float8_e4m3


def _host_prep(x, edge_index, W, att_src, att_dst, bias):
    src = np.asarray(edge_index[0], dtype=np.int64)
    dst = np.asarray(edge_index[1], dtype=np.int64)
    loop = np.arange(N, dtype=np.int64)
    src = np.concatenate([src, loop])
    dst = np.concatenate([dst, loop])

    deg = np.bincount(dst, minlength=N).astype(np.int64)

    # nodes sorted by degree desc -> global 128-slot blocks dealt round-robin
    # to cores so every core's j-th block has (nearly) equal max degree.
    order = np.argsort(-deg, kind="stable")
    ks = np.arange(NSLOT)
    g = ks // P
    p = ks % P
    c = g % NCORES
    j = g // NCORES
    rows = c * NPC + j * P + p          # device row of global sorted slot k
    row2node = np.full(NSLOT, -1, dtype=np.int64)
    row2node[rows[:N]] = order
    node2row = np.empty(N, dtype=np.int64)
    node2row[order] = rows[:N]

    # per-core-block uniform L schedule (exact max over the 8-block group)
    deg_slot = np.zeros(NSLOT, dtype=np.int64)
    deg_slot[:N] = deg[order]           # degree of global sorted slot k
    degb = deg_slot.reshape(NSLOT // P, P).max(axis=1)   # per global block g
    L_sched = degb.reshape(NB, NCORES).max(axis=1)
    L_sched = np.maximum(L_sched, 1)
    assert L_sched.max() <= 63, "a_src PSUM bank holds at most 63 edge columns"
    off = np.zeros(NB + 1, dtype=np.int64)
    off[1:] = np.cumsum(P * L_sched)
    S = int(off[-1])                    # edge slots per core

    # pad count per device row (for the analytic softmax-denominator fix)
    deg_row = np.zeros(NSLOT, dtype=np.int64)
    deg_row[rows] = deg_slot

    # folded weights
    Wt = np.asarray(W, dtype=np.float64).T            # [256, 64]
    att_s = np.asarray(att_src, np.float64)           # [8, 8]
    att_d = np.asarray(att_dst, np.float64)
    Wts = np.stack([Wt[:, h * C:(h + 1) * C] @ att_s[h] for h in range(H)], axis=1)  # [256, 8]
    Wtd = np.stack([Wt[:, h * C:(h + 1) * C] @ att_d[h] for h in range(H)], axis=1)  # [256, 8]

    def il(a, dtype):  # feature-interleave rows: [256, d] -> [128, 2, d]
        d = a.shape[1]
        return np.ascontiguousarray(
            a.reshape(2, P, d).transpose(1, 0, 2).astype(np.float32).astype(dtype))

    wf8 = il(Wt * SC_H, f8)            # [128, 2, 64] fp8, x4
    was8 = il(Wts * SC_A, f8)          # [128, 2, 8]  fp8, x4
    wdb = il(Wtd * SC_A, bf16)         # [128, 2, 8]  bf16, x4
    assert np.abs(Wt * SC_H).max() < 400 and np.abs(Wts * SC_A).max() < 400

    # DoubleRow identity: idr[p, i, m] = (m == p) for both k-tiles i
    idr = np.zeros((P, 2, P), dtype=f8)
    idr[np.arange(P), :, np.arange(P)] = 1.0

    # edge -> slot (vectorized); slot storage order (j, l, p)
    eorder = np.argsort(dst, kind="stable")
    dst_s = dst[eorder]
    src_s = src[eorder]
    starts = np.zeros(N + 1, dtype=np.int64)
    starts[1:] = np.cumsum(deg)
    l_rank = np.arange(len(dst_s), dtype=np.int64) - starts[dst_s]
    r = node2row[dst_s]
    ec = r // NPC
    within = r % NPC
    ej = within // P
    ep = within % P
    pos = off[ej] + l_rank * P + ep

    x_f8 = np.asarray(x, np.float32).astype(f8)
    assert np.abs(np.asarray(x, np.float32)).max() < 400  # e4m3 range

    bias_rep = np.tile(np.asarray(bias, np.float32).reshape(1, HC), (P, 1))

    in_maps = []
    for cc in range(NCORES):
        m = ec == cc
        xe = np.zeros((S, F), dtype=f8)               # pad slots stay zero
        xe[pos[m]] = x_f8[src_s[m]]
        # per block: [L, P, F] -> [F, L*P] -> k-half interleave [128, 2, L*P]
        parts = []
        for jj in range(NB):
            Lj = int(L_sched[jj])
            a = xe[off[jj]:off[jj + 1]].reshape(Lj, P, F)      # [l, p, f]
            a = a.transpose(2, 0, 1).reshape(2, P, Lj * P)     # [kh*128f, l*p]
            a = a.transpose(1, 0, 2)                           # [128f, kh, l*p]
            parts.append(np.ascontiguousarray(a).reshape(-1))
        xeT = np.concatenate(parts)
        del xe

        # own-node x, transposed + k-half interleaved, for a_dst
        rr = row2node[cc * NPC:(cc + 1) * NPC]
        mm = rr >= 0
        xo = np.zeros((NPC, F), dtype=f8)
        xo[mm] = x_f8[rr[mm]]
        xoT = np.ascontiguousarray(
            xo.T.reshape(2, P, NPC).transpose(1, 0, 2))        # [128, 2, NPC]

        # pad slots per row: [P, NB]
        d = deg_row[cc * NPC:(cc + 1) * NPC].reshape(NB, P)
        npad = (L_sched[:, None] - d).T.astype(np.float32).astype(bf16)

        in_maps.append({
            "xeT": xeT,
            "xoT": np.ascontiguousarray(xoT.reshape(P, 2 * NPC)),
            "wf8": wf8,
            "was8": was8,
            "wdb": wdb,
            "idr": np.ascontiguousarray(idr.reshape(P, 2 * P)),
            "bias_rep": bias_rep,
            "npad": np.ascontiguousarray(npad),
        })
    return in_maps, L_sched, S, row2node


def _build_program(L_sched, S, BX=5, BH=4, B2=3):
    nc = bacc.Bacc("TRN2", target_bir_lowering=False, debug=False,
                   enable_asserts=False, num_devices=NCORES)
    dt = mybir.dt
    DR = mybir.MatmulPerfMode.DoubleRow

    xeT = nc.dram_tensor("xeT", [S * 2 * P], dt.float8e4, kind="ExternalInput").ap()
    xoT = nc.dram_tensor("xoT", [P, 2 * NPC], dt.float8e4, kind="ExternalInput").ap()
    wf8 = nc.dram_tensor("wf8", [P, 2, HC], dt.float8e4, kind="ExternalInput").ap()
    was8 = nc.dram_tensor("was8", [P, 2, H], dt.float8e4, kind="ExternalInput").ap()
    wdb = nc.dram_tensor("wdb", [P, 2, H], dt.bfloat16, kind="ExternalInput").ap()
    idr = nc.dram_tensor("idr", [P, 2 * P], dt.float8e4, kind="ExternalInput").ap()
    bias_rep = nc.dram_tensor("bias_rep", [P, HC], dt.float32, kind="ExternalInput").ap()
    npad = nc.dram_tensor("npad", [P, NB], dt.bfloat16, kind="ExternalInput").ap()
    out = nc.dram_tensor("out", [P, NB * HC], dt.float16, kind="ExternalOutput").ap()

    AF = mybir.ActivationFunctionType
    OP = mybir.AluOpType
    GP8 = 8   # l-group: 8 x 64 fp32 fills one 2KB PSUM bank
    PB1 = 8   # phase-1: a_dst blocks per PSUM bank
    GB = 14   # finalize group: blocks normalized together, overlapping phase 2
    ISC = 1.0 / SC_A   # exp input scale undoing the x4 attention-weight scale

    with tile.TileContext(nc) as tc:
        with (
            tc.tile_pool(name="const", bufs=1) as constp,
            tc.tile_pool(name="resid", bufs=1) as residp,
        ):
            wf8_t = constp.tile([P, 2, HC], dt.float8e4)
            nc.sync.dma_start(wf8_t[:], wf8[:])
            was8_t = constp.tile([P, 2, H], dt.float8e4)
            nc.sync.dma_start(was8_t[:], was8[:])
            wdb_t = constp.tile([P, 2, H], dt.bfloat16)
            nc.sync.dma_start(wdb_t[:], wdb[:])
            idr_t = constp.tile([P, 2, P], dt.float8e4)
            nc.sync.dma_start(idr_t[:], idr[:].rearrange("p (k q) -> p k q", k=2))
            bias_t = constp.tile([P, HC], dt.float32)
            nc.sync.dma_start(bias_t[:], bias_rep[:])
            npad_t = constp.tile([P, NB], dt.bfloat16)
            nc.sync.dma_start(npad_t[:], npad[:])
            ident = constp.tile([P, P], dt.bfloat16)
            make_identity(nc, ident[:])
            xo_t = constp.tile([P, 2, NPC], dt.float8e4)
            nc.sync.dma_start(xo_t[:], xoT[:].rearrange("p (k q) -> p k q", k=2))

            adst_own = residp.tile([P, NB * H], dt.bfloat16)   # holds 4*a_dst
            padcorr = residp.tile([P, NB * H], dt.float32)
            aggsb = residp.tile([P, NB * TD], dt.float32)
            obuf = residp.tile([P, NB * HC], dt.float32)
            obuf16 = residp.tile([P, NB * HC], dt.float16)
            smbuf = residp.tile([P, NB], dt.float32)

            # ---------------- phase 1: a_dst + pad correction ----------------
            with (
                tc.tile_pool(name="p1", bufs=2) as p1,
                tc.tile_pool(name="p1ps", bufs=3, space="PSUM") as p1ps,
            ):
                for t0 in range(0, NB, PB1):
                    tn = min(PB1, NB - t0)
                    aps = p1ps.tile([P, tn * H], dt.float32, space="PSUM")
                    for ti in range(tn):
                        t = t0 + ti
                        nc.tensor.matmul(aps[:, ti * H:(ti + 1) * H],
                                         lhsT=xo_t[:, 0, t * P:(t + 1) * P],
                                         rhs=wdb_t[:, 0, :],
                                         start=(ti == 0), stop=False, skip_group_check=True)
                        nc.tensor.matmul(aps[:, ti * H:(ti + 1) * H],
                                         lhsT=xo_t[:, 1, t * P:(t + 1) * P],
                                         rhs=wdb_t[:, 1, :],
                                         start=False, stop=(ti == tn - 1), skip_group_check=True)
                    with nc.allow_low_precision(reason="bf16 a_dst store"):
                        nc.scalar.copy(out=adst_own[:, t0 * H:(t0 + tn) * H], in_=aps[:])
                # e_dst = exp(leaky(a_dst)); padcorr = npad * e_dst - eps
                e1 = p1.tile([P, NB * H], dt.bfloat16)
                nc.scalar.activation(e1[:], adst_own[:], AF.Exp, scale=ISC)
                e2 = p1.tile([P, NB * H], dt.bfloat16)
                nc.scalar.activation(e2[:], adst_own[:], AF.Exp, scale=NEG_SLOPE * ISC)
                edst = p1.tile([P, NB * H], dt.bfloat16)
                nc.vector.tensor_tensor(out=edst[:], in0=e1[:], in1=e2[:], op=OP.max)
                nc.vector.tensor_tensor(
                    out=padcorr[:].rearrange("p (t h) -> p t h", h=H),
                    in0=edst[:].rearrange("p (t h) -> p t h", h=H),
                    in1=npad_t[:].unsqueeze(2).to_broadcast([P, NB, H]),
                    op=OP.mult)
                nc.vector.tensor_scalar(out=padcorr[:], in0=padcorr[:],
                                        scalar1=1e-16, scalar2=None, op0=OP.subtract)

            # ---------------- phase 2: edge blocks ----------------
            with (
                tc.tile_pool(name="p2x", bufs=BX) as p2x,
                tc.tile_pool(name="p2", bufs=B2) as p2,
                tc.tile_pool(name="p2f", bufs=2) as p2f,
                tc.tile_pool(name="asrcps", bufs=2, space="PSUM") as asrcp,
                tc.tile_pool(name="heps", bufs=BH, space="PSUM") as hepsp,
                tc.tile_pool(name="aggps", bufs=2, space="PSUM") as aggpsp,
            ):
                xoff = 0
                for jb in range(NB):
                    L = int(L_sched[jb])
                    xta = p2x.tile([P, 2 * L * P], dt.float8e4, tag="xta")
                    nc.sync.dma_start(
                        xta[:], xeT[xoff:xoff + P * 2 * L * P].rearrange("(a b) -> a b", b=2 * L * P))
                    xoff += P * 2 * L * P
                    xv = xta[:].rearrange("p (k l q) -> p k l q", k=2, q=P)

                    # z = a_src + a_dst directly in one PSUM bank:
                    # prefill broadcast a_dst, then DoubleRow a_src matmuls
                    asrc = asrcp.tile([P, L * H], dt.float32, space="PSUM", tag="asrc")
                    nc.tensor.matmul(asrc[:], lhsT=xo_t[:, 0, jb * P:(jb + 1) * P],
                                     rhs=wdb_t[:, 0, :].unsqueeze(1).to_broadcast([P, L, H]),
                                     start=True, stop=False, skip_group_check=True)
                    nc.tensor.matmul(asrc[:], lhsT=xo_t[:, 1, jb * P:(jb + 1) * P],
                                     rhs=wdb_t[:, 1, :].unsqueeze(1).to_broadcast([P, L, H]),
                                     start=False, stop=False, skip_group_check=True)
                    for l in range(L):
                        nc.tensor.matmul(asrc[:, l * H:(l + 1) * H],
                                         lhsT=xv[:, :, l, :], rhs=was8_t[:],
                                         perf_mode=DR,
                                         start=False, stop=(l == L - 1), skip_group_check=True)
                    # e = max(exp(z/4), exp(0.2 z/4)) straight from PSUM
                    ex1 = p2.tile([P, L, H], dt.bfloat16, tag="ex1")
                    nc.scalar.activation(ex1[:], asrc[:].rearrange("p (l h) -> p l h", h=H),
                                         AF.Exp, scale=ISC)
                    ex2 = p2.tile([P, L, H], dt.bfloat16, tag="ex2")
                    nc.scalar.activation(ex2[:], asrc[:].rearrange("p (l h) -> p l h", h=H),
                                         AF.Exp, scale=NEG_SLOPE * ISC)
                    eb = p2.tile([P, L, H], dt.float8e4, tag="eb")
                    nc.vector.tensor_tensor(out=eb[:], in0=ex1[:], in1=ex2[:], op=OP.max)

                    agg = aggpsp.tile([P, TD], dt.float32, space="PSUM", tag="agg")
                    # e-part aggregation: agg[:, 64:72] += I.T @ e_l (DoubleRow pairs)
                    nl2 = L // 2
                    for i in range(nl2):
                        nc.tensor.matmul(agg[:, HC:TD], lhsT=idr_t[:],
                                         rhs=eb[:, 2 * i:2 * i + 2, :], perf_mode=DR,
                                         start=(i == 0), stop=False, skip_group_check=True)
                    if L % 2:
                        nc.tensor.matmul(agg[:, HC:TD], lhsT=ident[:], rhs=eb[:, L - 1, :],
                                         start=(nl2 == 0), stop=False, skip_group_check=True)
                    # h-part: per 8-edge group, DoubleRow h matmuls -> weight -> aggregate
                    for ch0 in range(0, L, GP8):
                        gl = min(GP8, L - ch0)
                        ps = hepsp.tile([P, gl * HC], dt.float32, space="PSUM", tag="heps")
                        for li in range(gl):
                            l = ch0 + li
                            nc.tensor.matmul(ps[:, li * HC:(li + 1) * HC],
                                             lhsT=xv[:, :, l, :], rhs=wf8_t[:],
                                             perf_mode=DR,
                                             start=(li == 0), stop=(li == gl - 1),
                                             skip_group_check=True)
                        w = p2.tile([P, gl, HC], dt.float8e4, tag="w")
                        nc.vector.tensor_tensor(
                            out=w[:].rearrange("p l (h c) -> p l h c", c=C),
                            in0=ps[:].rearrange("p (l h c) -> p l h c", h=H, c=C),
                            in1=eb[:, ch0:ch0 + gl, :].unsqueeze(3).to_broadcast([P, gl, H, C]),
                            op=OP.mult)
                        ng2 = gl // 2
                        for i in range(ng2):
                            l = ch0 + 2 * i
                            nc.tensor.matmul(agg[:, 0:HC], lhsT=idr_t[:],
                                             rhs=w[:, 2 * i:2 * i + 2, :], perf_mode=DR,
                                             start=False, stop=(l + 2 >= L), skip_group_check=True)
                        if gl % 2:
                            nc.tensor.matmul(agg[:, 0:HC], lhsT=ident[:], rhs=w[:, gl - 1, :],
                                             start=False, stop=(ch0 + gl >= L), skip_group_check=True)
                    # park [m | s] in SBUF; normalization runs in block groups
                    nc.scalar.copy(out=aggsb[:, jb * TD:(jb + 1) * TD], in_=agg[:])

                    # ---------------- grouped finalize (overlaps phase 2) ----------------
                    if (jb + 1) % GB == 0:
                        g0 = jb + 1 - GB
                        av = aggsb[:, g0 * TD:(jb + 1) * TD].rearrange("p (t d) -> p t d", d=TD)
                        pv = padcorr[:, g0 * H:(jb + 1) * H].rearrange("p (t h) -> p t h", h=H)
                        sden = p2f.tile([P, GB, H], dt.float32, tag="sden")
                        nc.gpsimd.tensor_tensor(out=sden[:], in0=av[:, :, HC:TD],
                                                in1=pv, op=OP.subtract)
                        srec = p2f.tile([P, GB, H], dt.float32, tag="srec")
                        nc.vector.reciprocal(srec[:], sden[:])
                        # undo the x4 h-weight scale here: onorm = m / (4 s)
                        srek = p2f.tile([P, GB, H], dt.float32, tag="srek")
                        nc.vector.tensor_scalar(out=srek[:], in0=srec[:],
                                                scalar1=1.0 / SC_H, scalar2=None, op0=OP.mult)
                        ov = obuf[:, g0 * HC:(jb + 1) * HC].rearrange("p (t d) -> p t d", d=HC)
                        nc.gpsimd.tensor_tensor(
                            out=ov.rearrange("p t (h c) -> p t h c", c=C),
                            in0=av[:, :, 0:HC].rearrange("p t (h c) -> p t h c", c=C),
                            in1=srek[:].unsqueeze(3).to_broadcast([P, GB, H, C]),
                            op=OP.mult)
                        nc.gpsimd.tensor_tensor(
                            out=ov, in0=ov,
                            in1=bias_t[:].unsqueeze(1).to_broadcast([P, GB, HC]), op=OP.add)
                        exf = p2f.tile([P, GB, HC], dt.bfloat16, tag="exf")
                        nc.scalar.activation(exf[:], ov, AF.Exp)
                        nc.vector.tensor_reduce(
                            smbuf[:, g0:jb + 1].unsqueeze(2), exf[:],
                            axis=mybir.AxisListType.X, op=OP.add)

                # ---------------- deferred log-softmax tail ----------------
                with tc.tile_pool(name="tail", bufs=1) as tailp:
                    lnb = tailp.tile([P, NB], dt.float32)
                    nc.scalar.activation(lnb[:], smbuf[:], AF.Ln)
                    with nc.allow_low_precision(reason="fp16 output"):
                        nc.vector.tensor_tensor(
                            out=obuf16[:].rearrange("p (t d) -> p t d", d=HC),
                            in0=obuf[:].rearrange("p (t d) -> p t d", d=HC),
                            in1=lnb[:].unsqueeze(2).to_broadcast([P, NB, HC]),
                            op=OP.subtract)
                    nc.sync.dma_start(out[:], obuf16[:])

    nc.compile()
    return nc


def kernel(x, edge_index, W, att_src, att_dst, bias):
    in_maps, L_sched, S, row2node = _host_prep(x, edge_index, W, att_src, att_dst, bias)
    nc = _build_program(L_sched, S)
    res = run_bass_kernel_spmd(nc, in_maps, core_ids=list(range(NCORES)))
    out_full = np.empty((N, HC), dtype=np.float32)
    for cc in range(NCORES):
        o = np.asarray(res.results[cc]["out"]).astype(np.float32)   # [128, NB*HC]
        o = o.reshape(P, NB, HC).transpose(1, 0, 2).reshape(NPC, HC)
        rr = row2node[cc * NPC:(cc + 1) * NPC]
        m = rr >= 0
        out_full[rr[m]] = o[m]
    return out_full


# revision 13
# speedup vs baseline: 2.9986x; 1.0036x over previous
"""GAT layer (PyG GATConv-style) on 8 Trainium2 NeuronCores.

Strategy (v5):
- Nodes sharded across 8 cores by destination; edges partitioned by destination
  node in a partition-aligned layout (degree-sorted bin packing, ~1.3% pad).
- Host expands x[src] per edge slot in fp8-e4m3 with a k-half-interleaved
  layout [pf][kh][l][p] that doubles as the DoubleRow matmul operand layout;
  per-edge h uses ONE DoubleRow fp8 matmul (k=256 in one pass, 0.5 cyc/row),
  as does per-edge a_src. Folded weights are fp8-e4m3, scaled x4 to dodge
  subnormals; the 1/4 is folded into the exp scale (attention) and the
  1/s normalization (h path).
- a_dst is pre-filled into the per-block a_src PSUM bank by a matmul with a
  broadcast-tiled Wtd rhs, so logits need no separate add; exps read PSUM.
- Padding slots are exact zeros; their softmax contribution is subtracted
  analytically (host pad-count x device exp(leaky(a_dst))).
- exp(leaky(z)) = max(exp(z), exp(0.2 z)) keeps ACT on one Exp table; the
  log-softmax finalize runs in 14-block groups overlapped under phase 2
  (Pool does the fp32 elementwise, ACT the exp and PSUM->SBUF moves), with
  only Ln + subtract + one fp16 output DMA at the end.
- Weighted aggregation: PSUM-accumulated DoubleRow identity matmuls over
  l-pairs (fp8 messages), bf16 identity for odd leftovers.

kernel(**inputs) takes FULL inputs and returns the FULL [N, 64] output.
"""

import numpy as np
import ml_dtypes

import concourse.bass as bass
import concourse.bacc as bacc
import concourse.tile as tile
from concourse import mybir
from concourse.bass_utils import run_bass_kernel_spmd
from concourse.masks import make_identity

# Problem shape (hardcoded per contract)
N, F, E = 100000, 256, 1600000
H, C = 8, 8
HC = H * C  # 64
NEG_SLOPE = 0.2
SC_H = 4.0   # fp8 weight scale for the h projection
SC_A = 4.0   # fp8/bf16 weight scale for the attention projections

P = 128
NCORES = 8
NB = 98                      # blocks per core
NPC = NB * P                 # 12544 node slots per core
NSLOT = NCORES * NPC         # 100352 >= N
TD = HC + H                  # 72: agg layout [h(64) | e(8)]

bf16 = ml_dtypes.bfloat16
f8 = ml_dtypes.float8_e4m3


def _host_prep(x, edge_index, W, att_src, att_dst, bias):
    src = np.asarray(edge_index[0], dtype=np.int64)
    dst = np.asarray(edge_index[1], dtype=np.int64)
    loop = np.arange(N, dtype=np.int64)
    src = np.concatenate([src, loop])
    dst = np.concatenate([dst, loop])

    deg = np.bincount(dst, minlength=N).astype(np.int64)

    # nodes sorted by degree desc -> global 128-slot blocks dealt round-robin
    # to cores so every core's j-th block has (nearly) equal max degree.
    order = np.argsort(-deg, kind="stable")
    ks = np.arange(NSLOT)
    g = ks // P
    p = ks % P
    c = g % NCORES
    j = g // NCORES
    rows = c * NPC + j * P + p          # device row of global sorted slot k
    row2node = np.full(NSLOT, -1, dtype=np.int64)
    row2node[rows[:N]] = order
    node2row = np.empty(N, dtype=np.int64)
    node2row[order] = rows[:N]

    # per-core-block uniform L schedule (exact max over the 8-block group)
    deg_slot = np.zeros(NSLOT, dtype=np.int64)
    deg_slot[:N] = deg[order]           # degree of global sorted slot k
    degb = deg_slot.reshape(NSLOT // P, P).max(axis=1)   # per global block g
    L_sched = degb.reshape(NB, NCORES).max(axis=1)
    L_sched = np.maximum(L_sched, 1)
    assert L_sched.max() <= 63, "a_src PSUM bank holds at most 63 edge columns"
    off = np.zeros(NB + 1, dtype=np.int64)
    off[1:] = np.cumsum(P * L_sched)
    S = int(off[-1])                    # edge slots per core

    # pad count per device row (for the analytic softmax-denominator fix)
    deg_row = np.zeros(NSLOT, dtype=np.int64)
    deg_row[rows] = deg_slot

    # folded weights
    Wt = np.asarray(W, dtype=np.float64).T            # [256, 64]
    att_s = np.asarray(att_src, np.float64)           # [8, 8]
    att_d = np.asarray(att_dst, np.float64)
    Wts = np.stack([Wt[:, h * C:(h + 1) * C] @ att_s[h] for h in range(H)], axis=1)  # [256, 8]
    Wtd = np.stack([Wt[:, h * C:(h + 1) * C] @ att_d[h] for h in range(H)], axis=1)  # [256, 8]

    def il(a, dtype):  # feature-interleave rows: [256, d] -> [128, 2, d]
        d = a.shape[1]
        return np.ascontiguousarray(
            a.reshape(2, P, d).transpose(1, 0, 2).astype(np.float32).astype(dtype))

    wf8 = il(Wt * SC_H, f8)            # [128, 2, 64] fp8, x4
    was8 = il(Wts * SC_A, f8)          # [128, 2, 8]  fp8, x4
    wdb = il(Wtd * SC_A, bf16)         # [128, 2, 8]  bf16, x4
    assert np.abs(Wt * SC_H).max() < 400 and np.abs(Wts * SC_A).max() < 400

    # DoubleRow identity: idr[p, i, m] = (m == p) for both k-tiles i
    idr = np.zeros((P, 2, P), dtype=f8)
    idr[np.arange(P), :, np.arange(P)] = 1.0

    # edge -> slot (vectorized); slot storage order (j, l, p)
    eorder = np.argsort(dst, kind="stable")
    dst_s = dst[eorder]
    src_s = src[eorder]
    starts = np.zeros(N + 1, dtype=np.int64)
    starts[1:] = np.cumsum(deg)
    l_rank = np.arange(len(dst_s), dtype=np.int64) - starts[dst_s]
    r = node2row[dst_s]
    ec = r // NPC
    within = r % NPC
    ej = within // P
    ep = within % P
    pos = off[ej] + l_rank * P + ep

    x_f8 = np.asarray(x, np.float32).astype(f8)
    assert np.abs(np.asarray(x, np.float32)).max() < 400  # e4m3 range

    bias_rep = np.tile(np.asarray(bias, np.float32).reshape(1, HC), (P, 1))

    in_maps = []
    for cc in range(NCORES):
        m = ec == cc
        xe = np.zeros((S, F), dtype=f8)               # pad slots stay zero
        xe[pos[m]] = x_f8[src_s[m]]
        # per block: [L, P, F] -> [F, L*P] -> k-half interleave [128, 2, L*P]
        parts = []
        for jj in range(NB):
            Lj = int(L_sched[jj])
            a = xe[off[jj]:off[jj + 1]].reshape(Lj, P, F)      # [l, p, f]
            a = a.transpose(2, 0, 1).reshape(2, P, Lj * P)     # [kh*128f, l*p]
            a = a.transpose(1, 0, 2)                           # [128f, kh, l*p]
            parts.append(np.ascontiguousarray(a).reshape(-1))
        xeT = np.concatenate(parts)
        del xe

        # own-node x, transposed + k-half interleaved, for a_dst
        rr = row2node[cc * NPC:(cc + 1) * NPC]
        mm = rr >= 0
        xo = np.zeros((NPC, F), dtype=f8)
        xo[mm] = x_f8[rr[mm]]
        xoT = np.ascontiguousarray(
            xo.T.reshape(2, P, NPC).transpose(1, 2, 0))        # [128, NPC, 2]

        # pad slots per row: [P, NB]
        d = deg_row[cc * NPC:(cc + 1) * NPC].reshape(NB, P)
        npad = (L_sched[:, None] - d).T.astype(np.float32).astype(bf16)

        in_maps.append({
            "xeT": xeT,
            "xoT": np.ascontiguousarray(xoT.reshape(P, 2 * NPC)),
            "wf8": wf8,
            "was8": was8,
            "wdb": wdb,
            "idr": np.ascontiguousarray(idr.reshape(P, 2 * P)),
            "bias_rep": bias_rep,
            "npad": np.ascontiguousarray(npad),
        })
    return in_maps, L_sched, S, row2node


def _build_program(L_sched, S, BX=5, BH=4, B2=4):
    nc = bacc.Bacc("TRN2", target_bir_lowering=False, debug=False,
                   enable_asserts=False, num_devices=NCORES)
    dt = mybir.dt
    DR = mybir.MatmulPerfMode.DoubleRow

    xeT = nc.dram_tensor("xeT", [S * 2 * P], dt.float8e4, kind="ExternalInput").ap()
    xoT = nc.dram_tensor("xoT", [P, 2 * NPC], dt.float8e4, kind="ExternalInput").ap()
    wf8 = nc.dram_tensor("wf8", [P, 2, HC], dt.float8e4, kind="ExternalInput").ap()
    was8 = nc.dram_tensor("was8", [P, 2, H], dt.float8e4, kind="ExternalInput").ap()
    wdb = nc.dram_tensor("wdb", [P, 2, H], dt.bfloat16, kind="ExternalInput").ap()
    idr = nc.dram_tensor("idr", [P, 2 * P], dt.float8e4, kind="ExternalInput").ap()
    bias_rep = nc.dram_tensor("bias_rep", [P, HC], dt.float32, kind="ExternalInput").ap()
    npad = nc.dram_tensor("npad", [P, NB], dt.bfloat16, kind="ExternalInput").ap()
    out = nc.dram_tensor("out", [P, NB * HC], dt.float16, kind="ExternalOutput").ap()

    AF = mybir.ActivationFunctionType
    OP = mybir.AluOpType
    GP8 = 8   # l-group: 8 x 64 fp32 fills one 2KB PSUM bank
    PB1 = 8   # phase-1: a_dst blocks per PSUM bank
    GB = 7    # finalize group: blocks normalized together, overlapping phase 2
    ISC = 1.0 / SC_A   # exp input scale undoing the x4 attention-weight scale

    with tile.TileContext(nc) as tc:
        with (
            tc.tile_pool(name="const", bufs=1) as constp,
            tc.tile_pool(name="resid", bufs=1) as residp,
        ):
            wf8_t = constp.tile([P, 2, HC], dt.float8e4)
            nc.sync.dma_start(wf8_t[:], wf8[:])
            was8_t = constp.tile([P, 2, H], dt.float8e4)
            nc.sync.dma_start(was8_t[:], was8[:])
            wdb_t = constp.tile([P, 2, H], dt.bfloat16)
            nc.sync.dma_start(wdb_t[:], wdb[:])
            idr_t = constp.tile([P, 2, P], dt.float8e4)
            nc.sync.dma_start(idr_t[:], idr[:].rearrange("p (k q) -> p k q", k=2))
            bias_t = constp.tile([P, HC], dt.float32)
            nc.sync.dma_start(bias_t[:], bias_rep[:])
            npad_t = constp.tile([P, NB], dt.bfloat16)
            nc.sync.dma_start(npad_t[:], npad[:])
            ident = constp.tile([P, P], dt.bfloat16)
            make_identity(nc, ident[:])
            xo_t = constp.tile([P, NPC, 2], dt.float8e4)
            QS = 2048   # early slice: unblocks phase-1/2 for the first 16 blocks
            nc.sync.dma_start(xo_t[:, 0:QS, :],
                              xoT[:, 0:2 * QS].rearrange("p (q k) -> p q k", k=2))
            nc.sync.dma_start(xo_t[:, QS:NPC, :],
                              xoT[:, 2 * QS:2 * NPC].rearrange("p (q k) -> p q k", k=2))

            adst_own = residp.tile([P, NB * H], dt.bfloat16)   # holds 4*a_dst
            padcorr = residp.tile([P, NB * H], dt.float32)
            aggsb = residp.tile([P, NB * TD], dt.float32)
            obuf = residp.tile([P, NB * HC], dt.float32)
            obuf16 = residp.tile([P, NB * HC], dt.float16)
            smbuf = residp.tile([P, NB], dt.float32)

            # ---------------- phase 1: a_dst + pad correction ----------------
            with (
                tc.tile_pool(name="p1", bufs=2) as p1,
                tc.tile_pool(name="p1ps", bufs=2, space="PSUM") as p1ps,
            ):
                for t0 in range(0, NB, PB1):
                    tn = min(PB1, NB - t0)
                    aps = p1ps.tile([P, tn * H], dt.float32, space="PSUM")
                    for ti in range(tn):
                        t = t0 + ti
                        nc.tensor.matmul(aps[:, ti * H:(ti + 1) * H],
                                         lhsT=xo_t[:, t * P:(t + 1) * P, 0],
                                         rhs=wdb_t[:, 0, :],
                                         start=(ti == 0), stop=False, skip_group_check=True)
                        nc.tensor.matmul(aps[:, ti * H:(ti + 1) * H],
                                         lhsT=xo_t[:, t * P:(t + 1) * P, 1],
                                         rhs=wdb_t[:, 1, :],
                                         start=False, stop=(ti == tn - 1), skip_group_check=True)
                    with nc.allow_low_precision(reason="bf16 a_dst store"):
                        nc.scalar.copy(out=adst_own[:, t0 * H:(t0 + tn) * H], in_=aps[:])
                # e_dst = exp(leaky(a_dst)); padcorr = npad * e_dst - eps
                e1 = p1.tile([P, NB * H], dt.bfloat16)
                nc.scalar.activation(e1[:], adst_own[:], AF.Exp, scale=ISC)
                e2 = p1.tile([P, NB * H], dt.bfloat16)
                nc.scalar.activation(e2[:], adst_own[:], AF.Exp, scale=NEG_SLOPE * ISC)
                edst = p1.tile([P, NB * H], dt.bfloat16)
                nc.vector.tensor_tensor(out=edst[:], in0=e1[:], in1=e2[:], op=OP.max)
                nc.vector.tensor_tensor(
                    out=padcorr[:].rearrange("p (t h) -> p t h", h=H),
                    in0=edst[:].rearrange("p (t h) -> p t h", h=H),
                    in1=npad_t[:].unsqueeze(2).to_broadcast([P, NB, H]),
                    op=OP.mult)
                nc.vector.tensor_scalar(out=padcorr[:], in0=padcorr[:],
                                        scalar1=1e-16, scalar2=None, op0=OP.subtract)

            # ---------------- phase 2: edge blocks ----------------
            with (
                tc.tile_pool(name="p2x", bufs=BX) as p2x,
                tc.tile_pool(name="p2", bufs=B2) as p2,
                tc.tile_pool(name="p2f", bufs=2) as p2f,
                tc.tile_pool(name="asrcps", bufs=2, space="PSUM") as asrcp,
                tc.tile_pool(name="heps", bufs=BH, space="PSUM") as hepsp,
                tc.tile_pool(name="aggps", bufs=2, space="PSUM") as aggpsp,
            ):
                xoff = 0
                for jb in range(NB):
                    L = int(L_sched[jb])
                    xta = p2x.tile([P, 2 * L * P], dt.float8e4, tag="xta")
                    nc.sync.dma_start(
                        xta[:], xeT[xoff:xoff + P * 2 * L * P].rearrange("(a b) -> a b", b=2 * L * P))
                    xoff += P * 2 * L * P
                    xv = xta[:].rearrange("p (k l q) -> p k l q", k=2, q=P)

                    # z = a_src + a_dst directly in one PSUM bank:
                    # prefill broadcast a_dst, then DoubleRow a_src matmuls
                    asrc = asrcp.tile([P, L * H], dt.float32, space="PSUM", tag="asrc")
                    nc.tensor.matmul(asrc[:], lhsT=xo_t[:, jb * P:(jb + 1) * P, 0],
                                     rhs=wdb_t[:, 0, :].unsqueeze(1).to_broadcast([P, L, H]),
                                     start=True, stop=False, skip_group_check=True)
                    nc.tensor.matmul(asrc[:], lhsT=xo_t[:, jb * P:(jb + 1) * P, 1],
                                     rhs=wdb_t[:, 1, :].unsqueeze(1).to_broadcast([P, L, H]),
                                     start=False, stop=False, skip_group_check=True)
                    for l in range(L):
                        nc.tensor.matmul(asrc[:, l * H:(l + 1) * H],
                                         lhsT=xv[:, :, l, :], rhs=was8_t[:],
                                         perf_mode=DR,
                                         start=False, stop=(l == L - 1), skip_group_check=True)
                    # e = max(exp(z/4), exp(0.2 z/4)) straight from PSUM
                    ex1 = p2.tile([P, L, H], dt.bfloat16, tag="ex1")
                    nc.scalar.activation(ex1[:], asrc[:].rearrange("p (l h) -> p l h", h=H),
                                         AF.Exp, scale=ISC)
                    ex2 = p2.tile([P, L, H], dt.bfloat16, tag="ex2")
                    nc.scalar.activation(ex2[:], asrc[:].rearrange("p (l h) -> p l h", h=H),
                                         AF.Exp, scale=NEG_SLOPE * ISC)
                    eb = p2.tile([P, L, H], dt.bfloat16, tag="eb")
                    nc.vector.tensor_tensor(out=eb[:], in0=ex1[:], in1=ex2[:], op=OP.max)

                    agg = aggpsp.tile([P, TD], dt.float32, space="PSUM", tag="agg")
                    # e-part aggregation: agg[:, 64:72] += I.T @ e_l
                    for l in range(L):
                        nc.tensor.matmul(agg[:, HC:TD], lhsT=ident[:], rhs=eb[:, l, :],
                                         start=(l == 0), stop=False, skip_group_check=True)
                    # h-part: per 8-edge group, DoubleRow h matmuls -> weight -> aggregate
                    for ch0 in range(0, L, GP8):
                        gl = min(GP8, L - ch0)
                        ps = hepsp.tile([P, gl * HC], dt.float32, space="PSUM", tag="heps")
                        for li in range(gl):
                            l = ch0 + li
                            nc.tensor.matmul(ps[:, li * HC:(li + 1) * HC],
                                             lhsT=xv[:, :, l, :], rhs=wf8_t[:],
                                             perf_mode=DR,
                                             start=(li == 0), stop=(li == gl - 1),
                                             skip_group_check=True)
                        w = p2.tile([P, gl, HC], dt.float8e4, tag="w")
                        nc.vector.tensor_tensor(
                            out=w[:].rearrange("p l (h c) -> p l h c", c=C),
                            in0=ps[:].rearrange("p (l h c) -> p l h c", h=H, c=C),
                            in1=eb[:, ch0:ch0 + gl, :].unsqueeze(3).to_broadcast([P, gl, H, C]),
                            op=OP.mult)
                        ng2 = gl // 2
                        for i in range(ng2):
                            l = ch0 + 2 * i
                            nc.tensor.matmul(agg[:, 0:HC], lhsT=idr_t[:],
                                             rhs=w[:, 2 * i:2 * i + 2, :], perf_mode=DR,
                                             start=False, stop=(l + 2 >= L), skip_group_check=True)
                        if gl % 2:
                            nc.tensor.matmul(agg[:, 0:HC], lhsT=ident[:], rhs=w[:, gl - 1, :],
                                             start=False, stop=(ch0 + gl >= L), skip_group_check=True)
                    # park [m | s] in SBUF; normalization runs in block groups
                    nc.scalar.copy(out=aggsb[:, jb * TD:(jb + 1) * TD], in_=agg[:])

                    # ---------------- grouped finalize (overlaps phase 2) ----------------
                    if (jb + 1) % GB == 0:
                        g0 = jb + 1 - GB
                        av = aggsb[:, g0 * TD:(jb + 1) * TD].rearrange("p (t d) -> p t d", d=TD)
                        pv = padcorr[:, g0 * H:(jb + 1) * H].rearrange("p (t h) -> p t h", h=H)
                        sden = p2f.tile([P, GB, H], dt.float32, tag="sden")
                        nc.gpsimd.tensor_tensor(out=sden[:], in0=av[:, :, HC:TD],
                                                in1=pv, op=OP.subtract)
                        srec = p2f.tile([P, GB, H], dt.float32, tag="srec")
                        nc.vector.reciprocal(srec[:], sden[:])
                        # undo the x4 h-weight scale here: onorm = m / (4 s)
                        srek = p2f.tile([P, GB, H], dt.float32, tag="srek")
                        nc.vector.tensor_scalar(out=srek[:], in0=srec[:],
                                                scalar1=1.0 / SC_H, scalar2=None, op0=OP.mult)
                        ov = obuf[:, g0 * HC:(jb + 1) * HC].rearrange("p (t d) -> p t d", d=HC)
                        nc.gpsimd.tensor_tensor(
                            out=ov.rearrange("p t (h c) -> p t h c", c=C),
                            in0=av[:, :, 0:HC].rearrange("p t (h c) -> p t h c", c=C),
                            in1=srek[:].unsqueeze(3).to_broadcast([P, GB, H, C]),
                            op=OP.mult)
                        nc.gpsimd.tensor_tensor(
                            out=ov, in0=ov,
                            in1=bias_t[:].unsqueeze(1).to_broadcast([P, GB, HC]), op=OP.add)
                        exf = p2f.tile([P, GB, HC], dt.bfloat16, tag="exf")
                        nc.scalar.activation(exf[:], ov, AF.Exp)
                        nc.vector.tensor_reduce(
                            smbuf[:, g0:jb + 1].unsqueeze(2), exf[:],
                            axis=mybir.AxisListType.X, op=OP.add)

                # ---------------- deferred log-softmax tail ----------------
                with tc.tile_pool(name="tail", bufs=1) as tailp:
                    lnb = tailp.tile([P, NB], dt.float32)
                    nc.scalar.activation(lnb[:], smbuf[:], AF.Ln)
                    with nc.allow_low_precision(reason="fp16 output"):
                        nc.vector.tensor_tensor(
                            out=obuf16[:].rearrange("p (t d) -> p t d", d=HC),
                            in0=obuf[:].rearrange("p (t d) -> p t d", d=HC),
                            in1=lnb[:].unsqueeze(2).to_broadcast([P, NB, HC]),
                            op=OP.subtract)
                    nc.sync.dma_start(out[:], obuf16[:])

    nc.compile()
    return nc


def kernel(x, edge_index, W, att_src, att_dst, bias):
    in_maps, L_sched, S, row2node = _host_prep(x, edge_index, W, att_src, att_dst, bias)
    nc = _build_program(L_sched, S)
    res = run_bass_kernel_spmd(nc, in_maps, core_ids=list(range(NCORES)))
    out_full = np.empty((N, HC), dtype=np.float32)
    for cc in range(NCORES):
        o = np.asarray(res.results[cc]["out"]).astype(np.float32)   # [128, NB*HC]
        o = o.reshape(P, NB, HC).transpose(1, 0, 2).reshape(NPC, HC)
        rr = row2node[cc * NPC:(cc + 1) * NPC]
        m = rr >= 0
        out_full[rr[m]] = o[m]
    return out_full


# revision 14
# speedup vs baseline: 3.0718x; 1.0244x over previous
"""GAT layer (PyG GATConv-style) on 8 Trainium2 NeuronCores.

Strategy (v5):
- Nodes sharded across 8 cores by destination; edges partitioned by destination
  node in a partition-aligned layout (degree-sorted bin packing, ~1.3% pad).
- Host expands x[src] per edge slot in fp8-e4m3 with a k-half-interleaved
  layout [pf][kh][l][p] that doubles as the DoubleRow matmul operand layout;
  per-edge h uses ONE DoubleRow fp8 matmul (k=256 in one pass, 0.5 cyc/row),
  as does per-edge a_src. Folded weights are fp8-e4m3, scaled x4 to dodge
  subnormals; the 1/4 is folded into the exp scale (attention) and the
  1/s normalization (h path).
- a_dst is pre-filled into the per-block a_src PSUM bank by a matmul with a
  broadcast-tiled Wtd rhs, so logits need no separate add; exps read PSUM.
- Padding slots are exact zeros; their softmax contribution is subtracted
  analytically (host pad-count x device exp(leaky(a_dst))).
- exp(leaky(z)) = max(exp(z), exp(0.2 z)) keeps ACT on one Exp table; the
  log-softmax finalize runs in 14-block groups overlapped under phase 2
  (Pool does the fp32 elementwise, ACT the exp and PSUM->SBUF moves), with
  only Ln + subtract + one fp16 output DMA at the end.
- Weighted aggregation: PSUM-accumulated DoubleRow identity matmuls over
  l-pairs (fp8 messages), bf16 identity for odd leftovers.

kernel(**inputs) takes FULL inputs and returns the FULL [N, 64] output.
"""

import numpy as np
import ml_dtypes

import concourse.bass as bass
import concourse.bacc as bacc
import concourse.tile as tile
from concourse import mybir
from concourse.bass_utils import run_bass_kernel_spmd
from concourse.masks import make_identity

# Problem shape (hardcoded per contract)
N, F, E = 100000, 256, 1600000
H, C = 8, 8
HC = H * C  # 64
NEG_SLOPE = 0.2
SC_H = 4.0   # fp8 weight scale for the h projection
SC_A = 4.0   # fp8/bf16 weight scale for the attention projections

P = 128
NCORES = 8
NB = 98                      # blocks per core
NPC = NB * P                 # 12544 node slots per core
NSLOT = NCORES * NPC         # 100352 >= N
TD = HC + H                  # 72: agg layout [h(64) | e(8)]

bf16 = ml_dtypes.bfloat16
f8 = ml_dtypes.float8_e4m3


def _host_prep(x, edge_index, W, att_src, att_dst, bias):
    src = np.asarray(edge_index[0], dtype=np.int64)
    dst = np.asarray(edge_index[1], dtype=np.int64)
    loop = np.arange(N, dtype=np.int64)
    src = np.concatenate([src, loop])
    dst = np.concatenate([dst, loop])

    deg = np.bincount(dst, minlength=N).astype(np.int64)

    # nodes sorted by degree desc -> global 128-slot blocks dealt round-robin
    # to cores so every core's j-th block has (nearly) equal max degree.
    order = np.argsort(-deg, kind="stable")
    ks = np.arange(NSLOT)
    g = ks // P
    p = ks % P
    c = g % NCORES
    j = g // NCORES
    rows = c * NPC + j * P + p          # device row of global sorted slot k
    row2node = np.full(NSLOT, -1, dtype=np.int64)
    row2node[rows[:N]] = order
    node2row = np.empty(N, dtype=np.int64)
    node2row[order] = rows[:N]

    # per-core-block uniform L schedule (exact max over the 8-block group)
    deg_slot = np.zeros(NSLOT, dtype=np.int64)
    deg_slot[:N] = deg[order]           # degree of global sorted slot k
    degb = deg_slot.reshape(NSLOT // P, P).max(axis=1)   # per global block g
    L_sched = degb.reshape(NB, NCORES).max(axis=1)
    L_sched = np.maximum(L_sched, 1)
    assert L_sched.max() <= 63, "a_src PSUM bank holds at most 63 edge columns"
    off = np.zeros(NB + 1, dtype=np.int64)
    off[1:] = np.cumsum(P * L_sched)
    S = int(off[-1])                    # edge slots per core

    # pad count per device row (for the analytic softmax-denominator fix)
    deg_row = np.zeros(NSLOT, dtype=np.int64)
    deg_row[rows] = deg_slot

    # folded weights
    Wt = np.asarray(W, dtype=np.float64).T            # [256, 64]
    att_s = np.asarray(att_src, np.float64)           # [8, 8]
    att_d = np.asarray(att_dst, np.float64)
    Wts = np.stack([Wt[:, h * C:(h + 1) * C] @ att_s[h] for h in range(H)], axis=1)  # [256, 8]
    Wtd = np.stack([Wt[:, h * C:(h + 1) * C] @ att_d[h] for h in range(H)], axis=1)  # [256, 8]

    def il(a, dtype):  # feature-interleave rows: [256, d] -> [128, 2, d]
        d = a.shape[1]
        return np.ascontiguousarray(
            a.reshape(2, P, d).transpose(1, 0, 2).astype(np.float32).astype(dtype))

    wf8 = il(Wt * SC_H, f8)            # [128, 2, 64] fp8, x4
    was8 = il(Wts * SC_A, f8)          # [128, 2, 8]  fp8, x4
    wdb = il(Wtd * SC_A, bf16)         # [128, 2, 8]  bf16, x4
    assert np.abs(Wt * SC_H).max() < 400 and np.abs(Wts * SC_A).max() < 400

    # DoubleRow identity: idr[p, i, m] = (m == p) for both k-tiles i
    idr = np.zeros((P, 2, P), dtype=f8)
    idr[np.arange(P), :, np.arange(P)] = 1.0

    # edge -> slot (vectorized); slot storage order (j, l, p)
    eorder = np.argsort(dst, kind="stable")
    dst_s = dst[eorder]
    src_s = src[eorder]
    starts = np.zeros(N + 1, dtype=np.int64)
    starts[1:] = np.cumsum(deg)
    l_rank = np.arange(len(dst_s), dtype=np.int64) - starts[dst_s]
    r = node2row[dst_s]
    ec = r // NPC
    within = r % NPC
    ej = within // P
    ep = within % P
    pos = off[ej] + l_rank * P + ep

    x_f8 = np.asarray(x, np.float32).astype(f8)
    assert np.abs(np.asarray(x, np.float32)).max() < 400  # e4m3 range

    bias_rep = np.tile(np.asarray(bias, np.float32).reshape(1, HC), (P, 1))

    in_maps = []
    for cc in range(NCORES):
        m = ec == cc
        xe = np.zeros((S, F), dtype=f8)               # pad slots stay zero
        xe[pos[m]] = x_f8[src_s[m]]
        # per block: [L, P, F] -> [F, L*P] -> k-half interleave [128, 2, L*P]
        parts = []
        for jj in range(NB):
            Lj = int(L_sched[jj])
            a = xe[off[jj]:off[jj + 1]].reshape(Lj, P, F)      # [l, p, f]
            a = a.transpose(2, 0, 1).reshape(2, P, Lj * P)     # [kh*128f, l*p]
            a = a.transpose(1, 0, 2)                           # [128f, kh, l*p]
            parts.append(np.ascontiguousarray(a).reshape(-1))
        xeT = np.concatenate(parts)
        del xe

        # own-node x, transposed + k-half interleaved, for a_dst
        rr = row2node[cc * NPC:(cc + 1) * NPC]
        mm = rr >= 0
        xo = np.zeros((NPC, F), dtype=f8)
        xo[mm] = x_f8[rr[mm]]
        xoT = np.ascontiguousarray(
            xo.T.reshape(2, P, NPC).transpose(1, 2, 0))        # [128, NPC, 2]

        # pad slots per row: [P, NB]
        d = deg_row[cc * NPC:(cc + 1) * NPC].reshape(NB, P)
        npad = (L_sched[:, None] - d).T.astype(np.float32).astype(bf16)

        in_maps.append({
            "xeT": xeT,
            "xoT": np.ascontiguousarray(xoT.reshape(P, 2 * NPC)),
            "wf8": wf8,
            "was8": was8,
            "wdb": wdb,
            "idr": np.ascontiguousarray(idr.reshape(P, 2 * P)),
            "bias_rep": bias_rep,
            "npad": np.ascontiguousarray(npad),
        })
    return in_maps, L_sched, S, row2node


def _build_program(L_sched, S, BX=5, BH=4, B2=4):
    nc = bacc.Bacc("TRN2", target_bir_lowering=False, debug=False,
                   enable_asserts=False, num_devices=NCORES)
    dt = mybir.dt
    DR = mybir.MatmulPerfMode.DoubleRow

    xeT = nc.dram_tensor("xeT", [S * 2 * P], dt.float8e4, kind="ExternalInput").ap()
    xoT = nc.dram_tensor("xoT", [P, 2 * NPC], dt.float8e4, kind="ExternalInput").ap()
    wf8 = nc.dram_tensor("wf8", [P, 2, HC], dt.float8e4, kind="ExternalInput").ap()
    was8 = nc.dram_tensor("was8", [P, 2, H], dt.float8e4, kind="ExternalInput").ap()
    wdb = nc.dram_tensor("wdb", [P, 2, H], dt.bfloat16, kind="ExternalInput").ap()
    idr = nc.dram_tensor("idr", [P, 2 * P], dt.float8e4, kind="ExternalInput").ap()
    bias_rep = nc.dram_tensor("bias_rep", [P, HC], dt.float32, kind="ExternalInput").ap()
    npad = nc.dram_tensor("npad", [P, NB], dt.bfloat16, kind="ExternalInput").ap()
    out = nc.dram_tensor("out", [P, NB * HC], dt.float16, kind="ExternalOutput").ap()

    AF = mybir.ActivationFunctionType
    OP = mybir.AluOpType
    GP8 = 8   # l-group: 8 x 64 fp32 fills one 2KB PSUM bank
    PB1 = 64  # phase-1: a_dst blocks per PSUM bank (512 fp32 = full bank)
    GB = 7    # finalize group: blocks normalized together, overlapping phase 2
    NCUT = (NB // GB - 1) * GB   # early-flush boundary: all but the last group
    ISC = 1.0 / SC_A   # exp input scale undoing the x4 attention-weight scale

    with tile.TileContext(nc) as tc:
        with (
            tc.tile_pool(name="const", bufs=1) as constp,
            tc.tile_pool(name="resid", bufs=1) as residp,
        ):
            wf8_t = constp.tile([P, 2, HC], dt.float8e4)
            nc.sync.dma_start(wf8_t[:], wf8[:])
            was8_t = constp.tile([P, 2, H], dt.float8e4)
            nc.sync.dma_start(was8_t[:], was8[:])
            wdb_t = constp.tile([P, 2, H], dt.bfloat16)
            nc.sync.dma_start(wdb_t[:], wdb[:])
            idr_t = constp.tile([P, 2, P], dt.float8e4)
            nc.sync.dma_start(idr_t[:], idr[:].rearrange("p (k q) -> p k q", k=2))
            bias_t = constp.tile([P, HC], dt.float32)
            nc.sync.dma_start(bias_t[:], bias_rep[:])
            npad_t = constp.tile([P, NB], dt.bfloat16)
            nc.sync.dma_start(npad_t[:], npad[:])
            ident = constp.tile([P, P], dt.bfloat16)
            make_identity(nc, ident[:])
            xo_t = constp.tile([P, NPC, 2], dt.float8e4)
            QS = 2048   # early slice: unblocks phase-1/2 for the first 16 blocks
            nc.sync.dma_start(xo_t[:, 0:QS, :],
                              xoT[:, 0:2 * QS].rearrange("p (q k) -> p q k", k=2))
            nc.sync.dma_start(xo_t[:, QS:NPC, :],
                              xoT[:, 2 * QS:2 * NPC].rearrange("p (q k) -> p q k", k=2))

            adst_own = residp.tile([P, NB * H], dt.bfloat16)   # holds 4*a_dst
            padcorr = residp.tile([P, NB * H], dt.float32)
            aggsb = residp.tile([P, NB * TD], dt.float32)
            obuf = residp.tile([P, NB * HC], dt.float32)
            obuf16 = residp.tile([P, NB * HC], dt.float16)
            smbuf = residp.tile([P, NB], dt.float32)
            lnb_t = residp.tile([P, NB], dt.float32)

            # ---------------- phase 1: a_dst + pad correction ----------------
            with (
                tc.tile_pool(name="p1", bufs=2) as p1,
                tc.tile_pool(name="p1ps", bufs=2, space="PSUM") as p1ps,
            ):
                for t0 in range(0, NB, PB1):
                    tn = min(PB1, NB - t0)
                    aps = p1ps.tile([P, tn * H], dt.float32, space="PSUM")
                    for ti in range(tn):
                        t = t0 + ti
                        nc.tensor.matmul(aps[:, ti * H:(ti + 1) * H],
                                         lhsT=xo_t[:, t * P:(t + 1) * P, 0],
                                         rhs=wdb_t[:, 0, :],
                                         start=(ti == 0), stop=False, skip_group_check=True)
                        nc.tensor.matmul(aps[:, ti * H:(ti + 1) * H],
                                         lhsT=xo_t[:, t * P:(t + 1) * P, 1],
                                         rhs=wdb_t[:, 1, :],
                                         start=False, stop=(ti == tn - 1), skip_group_check=True)
                    with nc.allow_low_precision(reason="bf16 a_dst store"):
                        nc.scalar.copy(out=adst_own[:, t0 * H:(t0 + tn) * H], in_=aps[:])
                # e_dst = exp(leaky(a_dst)); padcorr = npad * e_dst - eps
                e1 = p1.tile([P, NB * H], dt.bfloat16)
                nc.scalar.activation(e1[:], adst_own[:], AF.Exp, scale=ISC)
                e2 = p1.tile([P, NB * H], dt.bfloat16)
                nc.scalar.activation(e2[:], adst_own[:], AF.Exp, scale=NEG_SLOPE * ISC)
                edst = p1.tile([P, NB * H], dt.bfloat16)
                nc.vector.tensor_tensor(out=edst[:], in0=e1[:], in1=e2[:], op=OP.max)
                nc.vector.tensor_tensor(
                    out=padcorr[:].rearrange("p (t h) -> p t h", h=H),
                    in0=edst[:].rearrange("p (t h) -> p t h", h=H),
                    in1=npad_t[:].unsqueeze(2).to_broadcast([P, NB, H]),
                    op=OP.mult)
                nc.vector.tensor_scalar(out=padcorr[:], in0=padcorr[:],
                                        scalar1=1e-16, scalar2=None, op0=OP.subtract)

            # ---------------- phase 2: edge blocks ----------------
            with (
                tc.tile_pool(name="p2x", bufs=BX) as p2x,
                tc.tile_pool(name="p2", bufs=B2) as p2,
                tc.tile_pool(name="p2f", bufs=2) as p2f,
                tc.tile_pool(name="asrcps", bufs=2, space="PSUM") as asrcp,
                tc.tile_pool(name="heps", bufs=BH, space="PSUM") as hepsp,
                tc.tile_pool(name="aggps", bufs=2, space="PSUM") as aggpsp,
            ):
                xoff = 0
                for jb in range(NB):
                    L = int(L_sched[jb])
                    xta = p2x.tile([P, 2 * L * P], dt.float8e4, tag="xta")
                    nc.sync.dma_start(
                        xta[:], xeT[xoff:xoff + P * 2 * L * P].rearrange("(a b) -> a b", b=2 * L * P))
                    xoff += P * 2 * L * P
                    xv = xta[:].rearrange("p (k l q) -> p k l q", k=2, q=P)

                    # z = a_src + a_dst directly in one PSUM bank:
                    # prefill broadcast a_dst, then DoubleRow a_src matmuls
                    asrc = asrcp.tile([P, L * H], dt.float32, space="PSUM", tag="asrc")
                    nc.tensor.matmul(asrc[:], lhsT=xo_t[:, jb * P:(jb + 1) * P, 0],
                                     rhs=wdb_t[:, 0, :].unsqueeze(1).to_broadcast([P, L, H]),
                                     start=True, stop=False, skip_group_check=True)
                    nc.tensor.matmul(asrc[:], lhsT=xo_t[:, jb * P:(jb + 1) * P, 1],
                                     rhs=wdb_t[:, 1, :].unsqueeze(1).to_broadcast([P, L, H]),
                                     start=False, stop=False, skip_group_check=True)
                    for l in range(L):
                        nc.tensor.matmul(asrc[:, l * H:(l + 1) * H],
                                         lhsT=xv[:, :, l, :], rhs=was8_t[:],
                                         perf_mode=DR,
                                         start=False, stop=(l == L - 1), skip_group_check=True)
                    # e = max(exp(z/4), exp(0.2 z/4)) straight from PSUM
                    ex1 = p2.tile([P, L, H], dt.bfloat16, tag="ex1")
                    nc.scalar.activation(ex1[:], asrc[:].rearrange("p (l h) -> p l h", h=H),
                                         AF.Exp, scale=ISC)
                    ex2 = p2.tile([P, L, H], dt.bfloat16, tag="ex2")
                    nc.scalar.activation(ex2[:], asrc[:].rearrange("p (l h) -> p l h", h=H),
                                         AF.Exp, scale=NEG_SLOPE * ISC)
                    eb = p2.tile([P, L, H], dt.bfloat16, tag="eb")
                    nc.vector.tensor_tensor(out=eb[:], in0=ex1[:], in1=ex2[:], op=OP.max)

                    agg = aggpsp.tile([P, TD], dt.float32, space="PSUM", tag="agg")
                    # e-part aggregation: agg[:, 64:72] += I.T @ e_l
                    for l in range(L):
                        nc.tensor.matmul(agg[:, HC:TD], lhsT=ident[:], rhs=eb[:, l, :],
                                         start=(l == 0), stop=False, skip_group_check=True)
                    # h-part: per 8-edge group, DoubleRow h matmuls -> weight -> aggregate
                    for ch0 in range(0, L, GP8):
                        gl = min(GP8, L - ch0)
                        ps = hepsp.tile([P, gl * HC], dt.float32, space="PSUM", tag="heps")
                        for li in range(gl):
                            l = ch0 + li
                            nc.tensor.matmul(ps[:, li * HC:(li + 1) * HC],
                                             lhsT=xv[:, :, l, :], rhs=wf8_t[:],
                                             perf_mode=DR,
                                             start=(li == 0), stop=(li == gl - 1),
                                             skip_group_check=True)
                        w = p2.tile([P, gl, HC], dt.float8e4, tag="w")
                        nc.vector.tensor_tensor(
                            out=w[:].rearrange("p l (h c) -> p l h c", c=C),
                            in0=ps[:].rearrange("p (l h c) -> p l h c", h=H, c=C),
                            in1=eb[:, ch0:ch0 + gl, :].unsqueeze(3).to_broadcast([P, gl, H, C]),
                            op=OP.mult)
                        ng2 = gl // 2
                        for i in range(ng2):
                            l = ch0 + 2 * i
                            nc.tensor.matmul(agg[:, 0:HC], lhsT=idr_t[:],
                                             rhs=w[:, 2 * i:2 * i + 2, :], perf_mode=DR,
                                             start=False, stop=(l + 2 >= L), skip_group_check=True)
                        if gl % 2:
                            nc.tensor.matmul(agg[:, 0:HC], lhsT=ident[:], rhs=w[:, gl - 1, :],
                                             start=False, stop=(ch0 + gl >= L), skip_group_check=True)
                    # park [m | s] in SBUF; normalization runs in block groups
                    nc.scalar.copy(out=aggsb[:, jb * TD:(jb + 1) * TD], in_=agg[:])

                    # ---------------- grouped finalize (overlaps phase 2) ----------------
                    if (jb + 1) % GB == 0:
                        g0 = jb + 1 - GB
                        av = aggsb[:, g0 * TD:(jb + 1) * TD].rearrange("p (t d) -> p t d", d=TD)
                        pv = padcorr[:, g0 * H:(jb + 1) * H].rearrange("p (t h) -> p t h", h=H)
                        sden = p2f.tile([P, GB, H], dt.float32, tag="sden")
                        nc.gpsimd.tensor_tensor(out=sden[:], in0=av[:, :, HC:TD],
                                                in1=pv, op=OP.subtract)
                        srec = p2f.tile([P, GB, H], dt.float32, tag="srec")
                        nc.vector.reciprocal(srec[:], sden[:])
                        # undo the x4 h-weight scale here: onorm = m / (4 s)
                        srek = p2f.tile([P, GB, H], dt.float32, tag="srek")
                        nc.vector.tensor_scalar(out=srek[:], in0=srec[:],
                                                scalar1=1.0 / SC_H, scalar2=None, op0=OP.mult)
                        ov = obuf[:, g0 * HC:(jb + 1) * HC].rearrange("p (t d) -> p t d", d=HC)
                        nc.gpsimd.tensor_tensor(
                            out=ov.rearrange("p t (h c) -> p t h c", c=C),
                            in0=av[:, :, 0:HC].rearrange("p t (h c) -> p t h c", c=C),
                            in1=srek[:].unsqueeze(3).to_broadcast([P, GB, H, C]),
                            op=OP.mult)
                        nc.gpsimd.tensor_tensor(
                            out=ov, in0=ov,
                            in1=bias_t[:].unsqueeze(1).to_broadcast([P, GB, HC]), op=OP.add)
                        exf = p2f.tile([P, GB, HC], dt.bfloat16, tag="exf")
                        nc.scalar.activation(exf[:], ov, AF.Exp)
                        nc.vector.tensor_reduce(
                            smbuf[:, g0:jb + 1].unsqueeze(2), exf[:],
                            axis=mybir.AxisListType.X, op=OP.add)

                    # early log-softmax flush: everything finalized so far
                    # ships while the last blocks still compute
                    if jb == NCUT - 1:
                        nc.scalar.activation(lnb_t[:, 0:NCUT], smbuf[:, 0:NCUT], AF.Ln)
                        with nc.allow_low_precision(reason="fp16 output"):
                            nc.vector.tensor_tensor(
                                out=obuf16[:, 0:NCUT * HC].rearrange("p (t d) -> p t d", d=HC),
                                in0=obuf[:, 0:NCUT * HC].rearrange("p (t d) -> p t d", d=HC),
                                in1=lnb_t[:, 0:NCUT].unsqueeze(2).to_broadcast([P, NCUT, HC]),
                                op=OP.subtract)
                        nc.sync.dma_start(out[:, 0:NCUT * HC], obuf16[:, 0:NCUT * HC])

                # ---------------- deferred log-softmax tail (last groups) ----------------
                NREM = NB - NCUT
                nc.scalar.activation(lnb_t[:, NCUT:NB], smbuf[:, NCUT:NB], AF.Ln)
                with nc.allow_low_precision(reason="fp16 output"):
                    nc.vector.tensor_tensor(
                        out=obuf16[:, NCUT * HC:].rearrange("p (t d) -> p t d", d=HC),
                        in0=obuf[:, NCUT * HC:].rearrange("p (t d) -> p t d", d=HC),
                        in1=lnb_t[:, NCUT:NB].unsqueeze(2).to_broadcast([P, NREM, HC]),
                        op=OP.subtract)
                nc.sync.dma_start(out[:, NCUT * HC:], obuf16[:, NCUT * HC:])

    nc.compile()
    return nc


def kernel(x, edge_index, W, att_src, att_dst, bias):
    in_maps, L_sched, S, row2node = _host_prep(x, edge_index, W, att_src, att_dst, bias)
    nc = _build_program(L_sched, S)
    res = run_bass_kernel_spmd(nc, in_maps, core_ids=list(range(NCORES)))
    out_full = np.empty((N, HC), dtype=np.float32)
    for cc in range(NCORES):
        o = np.asarray(res.results[cc]["out"]).astype(np.float32)   # [128, NB*HC]
        o = o.reshape(P, NB, HC).transpose(1, 0, 2).reshape(NPC, HC)
        rr = row2node[cc * NPC:(cc + 1) * NPC]
        m = rr >= 0
        out_full[rr[m]] = o[m]
    return out_full
